# revision 88
# baseline (speedup 1.0000x reference)
"""TRN2 Bass kernel for fused MHA (softmax-over-query quirk) + out-proj + residual + LayerNorm.

Problem shapes (hardcoded): tokens [4,2048,1024], Wq/Wk [16,1024,64], Wv [16,1024,64],
Wo [1024,1024], gamma/beta [1024]. Output [4,2048,1024] fp32.

Sharding: 8 cores, core c owns (batch b=c//2, S-half jc=c%2) of the OUTPUT rows.
No collectives. Each core computes, for its batch b:
  qT[dk,i] (full S), kT[dk,j] (its half) in bf16, V[i,dv] in fp8 (x64),
  scores^T[i,j] = q_i.k_j (PSUM fp32), e = exp(scores/8) in fp8e4m3,
  heads^T[dv,j] + rowsum row via a x64 ones-column appended to V,
  multi^T = heads^T / rowsum, out = multi @ Wo + tokens, LayerNorm rows.

All four projections AND the attention-value contraction run in fp8e4m3
DoubleRow (2 K-planes per matmul): tokens/multi cast to fp8, weights x256
(dodges e4m3 subnormals; the scales cancel exactly through softmax and the
out-proj's 2^8 is divided out in the residual-add STT). attnV pairs two
i-chunks per pass: each exp writes one plane of a DoubleRow-ready
[128, 2, 512] fp8 e-tile, V is stored [i, ic, h, dv|ones] so v_sb[:,2m:2m+2,h,:]
is the matching [128,2,65] stationary block -- attnV matmul count halves.

The exp stream (the former bottleneck) is split across engines: hh0 exps on
Scalar (Exp -> fp8 out, same cost as bf16), hh1 exps on the DVE for 3 of 4
ic-pairs via the e4m3 bit-trick int8(round(x*8*log2e + 56)) bitcast to fp8
-- one tensor_scalar op whose int8 write-saturation doubles as an underflow
clamp (bits -128 = -0.0). The mod-4 pattern keeps each block's last pair
all-Scalar so block-boundary deps never queue behind the DVE.

Schedule: pair-major blocks (pr, sweep) x 16 i-chunks; projections ride as
hooks in iteration slots (each pair's j-half-1 block carries the next
pair's q/k chains and V projections). attnV pops trail their exps by 6-7
ic-PAIRS (deep lag: exp latency/queueing never stalls the PE; psAcc's two
banks stay consistent with pops draining one block late). Normalize is
split: pop-time stage1 (acc->SBUF drain + rowsum DRAM-broadcast roundtrip),
stage2 (reciprocal + GpSimd mults) deferred 3 iterations so the in-order
DVE queue never parks on the roundtrip. Matmuls are NOT chained (the
scheduler fills stalls). DRAM layouts are block-major so every DMA is
contiguous per partition. Phase C: fp8-DR out-proj with 4+4 PSUM prefill
waves, residual+LN with variance on Scalar and the final scale alternating
Scalar/DVE. Measured ~325us on 8 cores (prior session 380us, stub 513us),
rel err 8.2e-4 (gate 2e-2).
"""

import numpy as np
import ml_dtypes

BF16 = ml_dtypes.bfloat16
FP8 = ml_dtypes.float8_e4m3

B, S, D, H, DK, DV = 4, 2048, 1024, 16, 64, 64
NCORES = 8
NPAIR = 8     # head pairs
NKC = 8       # D // 128 contraction chunks
NIC = 16      # S // 128 i-chunks
JW = 1024     # j columns per core (S/2)
NJCH = 8      # JW // 128
LN_EPS = 1e-5
WSCALE = 256.0  # fp8 weight pre-scale (power of 2)
VSCALE = 64.0   # on-chip V scale: psum(x256) * 0.25 -> fp8 with no clipping
EXP_SCALE = 0.125 / (WSCALE * WSCALE)  # true scores/8 per PSUM unit
LOG2E = 1.4426950408889634
# DVE exp bit-trick: int8(round(x*EXP_A + EXP_B)) viewed as e4m3 ~= exp(x*EXP_SCALE)
EXP_A = 8.0 * LOG2E * EXP_SCALE
EXP_B = 56.0  # e4m3 exponent bias 7 << 3 mantissa bits
DVE_PAIR_MOD = 4   # of every 4 ic-pairs, this many run their hh1 exp on the DVE
DVE_PAIR_CNT = 3   # (hh0 always on ACT: split pairs have latency max, not sum)

_CACHE = {}


def _build_nc(apply_affine):
    import concourse.tile as tile
    from concourse import bacc, mybir

    F32 = mybir.dt.float32
    BF = mybir.dt.bfloat16
    F8 = mybir.dt.float8e4
    I8 = mybir.dt.int8
    Exp = mybir.ActivationFunctionType.Exp
    Copy = mybir.ActivationFunctionType.Copy
    Square = mybir.ActivationFunctionType.Square
    Sqrt = mybir.ActivationFunctionType.Sqrt
    mult = mybir.AluOpType.mult
    add = mybir.AluOpType.add
    DR = mybir.MatmulPerfMode.DoubleRow

    nc = bacc.Bacc(
        "TRN2",
        target_bir_lowering=False,
        debug=False,
        enable_asserts=False,
        num_devices=NCORES,
    )

    # DRAM I/O (per-core views; host prepares layouts). Block-major orders
    # (i-blocks of 512 for tokens, head-pair blocks for Wq/Wk, head-half
    # blocks for Wv/Wo) make every DMA the kernel issues contiguous per
    # partition -- high descriptor efficiency, fast startup ramp.
    tokT_d = nc.dram_tensor("tokT", (128, 4, NKC, 512), F8, kind="ExternalInput").ap()
    tokTj_d = nc.dram_tensor("tokTj", (128, 2, NKC, 512), F8, kind="ExternalInput").ap()
    wq_d = nc.dram_tensor("wq", (128, NPAIR, NKC, 128), F8, kind="ExternalInput").ap()
    wk_d = nc.dram_tensor("wk", (128, NPAIR, NKC, 128), F8, kind="ExternalInput").ap()
    wv_d = nc.dram_tensor("wv", (128, 2, NKC, 512), F8, kind="ExternalInput").ap()
    wo_d = nc.dram_tensor("wo", (128, 2, NKC, 512), F8, kind="ExternalInput").ap()
    tokres_d = nc.dram_tensor("tokres", (128, NJCH, D), F32, kind="ExternalInput").ap()
    if apply_affine:
        gamma_d = nc.dram_tensor("gamma_bc", (128, D), F32, kind="ExternalInput").ap()
        beta_d = nc.dram_tensor("beta_bc", (128, D), F32, kind="ExternalInput").ap()
    out_d = nc.dram_tensor("out", (128, NJCH, D), F32, kind="ExternalOutput").ap()
    from contextlib import ExitStack

    from concourse.bass import _add_dep_helper

    # Chain all PE matmuls in emission order: stops the scheduler from
    # interleaving row-conflicting matmuls and keeps the stream dense.
    CHAIN_MM = False
    _prev_mm = [None]

    def mm(*args, **kwargs):
        inst = nc.tensor.matmul(*args, **kwargs)
        if CHAIN_MM and _prev_mm[0] is not None:
            _add_dep_helper(inst.ins, _prev_mm[0].ins, sync=False, reason="pe-order")
        _prev_mm[0] = inst
        return inst

    with tile.TileContext(nc) as tc, ExitStack() as stack:
        persist = stack.enter_context(tc.tile_pool(name="persist", bufs=1))
        qT_sb = persist.tile([128, NPAIR, S], BF)          # [pair-dk, pr, i]
        kT_sb = persist.tile([128, NPAIR, JW], BF)         # [pair-dk, pr, j]
        v_sb = persist.tile([128, NIC, H, DV + 1], F8)     # [i%128, ic, h, dv|64s]
        # multi^T in fp8, one tile per KC-PAIR ([128, 2, JW], middle dim = the
        # DoubleRow K-plane) so the out-proj runs fp8 DoubleRow; per-pair-ish
        # tiles keep out-proj dep tracking from serializing on the last write
        multiT = [
            persist.tile([128, 2, JW], F8, name=f"multiT{i}") for i in range(NKC // 2)
        ]
        eps_sb = persist.tile([128, 1], F32)
        if apply_affine:
            gamma_sb = persist.tile([128, D], F32)
            beta_sb = persist.tile([128, D], F32)
            nc.sync.dma_start(gamma_sb[:], gamma_d[:])
            nc.sync.dma_start(beta_sb[:], beta_d[:])
        nc.vector.memset(eps_sb[:], LN_EPS)
        for ic in range(NIC):
            # ones column scaled by VSCALE so rowsum matches the x64 V scale
            nc.vector.memset(v_sb[:, ic, :, DV : DV + 1], VSCALE)

        # pools that outlive pa must be allocated first (LIFO release)
        # psS: six 1-bank [128,512] slots shared by the scores ring and the
        # projection-chain borrows -- deep enough that neither a scores matmul
        # nor a proj chain ever waits on a slot whose exp hasn't fired yet.
        psS = tc.alloc_tile_pool(name="psS", bufs=5, space="PSUM")
        psP = tc.alloc_tile_pool(name="psP", bufs=1, space="PSUM")
        psAcc = tc.alloc_tile_pool(name="psAcc", bufs=2, space="PSUM")
        pe_pool = stack.enter_context(tc.tile_pool(name="pe", bufs=22))
        pn_pool = stack.enter_context(tc.tile_pool(name="pn", bufs=6))
        pdram = stack.enter_context(tc.tile_pool(name="pdram", bufs=6, space="DRAM"))

        # tokTj and wk outlive pa (the k j-half-1 chains consume them in
        # sweep 1), so they live in their own right-side pool.
        paJ = tc.alloc_tile_pool(name="paJ", bufs=1, side="right")
        tokTj_sb = paJ.tile([128, 2, NKC, 512], F8)
        wk_sb = paJ.tile([128, NPAIR, NKC, 128], F8)
        pa = tc.alloc_tile_pool(name="pa", bufs=1)
        wq_sb = pa.tile([128, NPAIR, NKC, 128], F8)
        tokT_sb = pa.tile([128, 4, NKC, 512], F8)
        wv_sb = pa.tile([128, 2, NKC, 512], F8)

        # Startup DMA over both HWDGE rings (SP + ACT, independent FIFOs).
        # Every transfer below is contiguous per partition. Critical path to
        # the first exp: wq pair-0 + tokens i-block-0 + wk pair-0 + tokTj
        # j-half-0; everything else queues behind.
        nc.sync.dma_start(wq_sb[:, 0], wq_d[:, 0])
        nc.scalar.dma_start(tokT_sb[:, 0], tokT_d[:, 0])
        nc.sync.dma_start(wk_sb[:, 0], wk_d[:, 0])
        nc.sync.dma_start(tokTj_sb[:, 0], tokTj_d[:, 0])
        nc.scalar.dma_start(wv_sb[:], wv_d[:])
        nc.sync.dma_start(tokTj_sb[:, 1], tokTj_d[:, 1])
        nc.scalar.dma_start(tokT_sb[:, 1], tokT_d[:, 1])
        nc.sync.dma_start(tokT_sb[:, 2], tokT_d[:, 2])
        nc.sync.dma_start(tokT_sb[:, 3], tokT_d[:, 3])
        nc.sync.dma_start(wq_sb[:, 1:], wq_d[:, 1:])
        nc.sync.dma_start(wk_sb[:, 1:], wk_d[:, 1:])

        def proj_chain(pr, which, t):
            """One 512-wide fp8 DoubleRow projection chain via a borrowed
            scores-pool slot."""
            w_sb, dst, rhs_sb = (
                (wq_sb, qT_sb, tokT_sb) if which == "q" else (wk_sb, kT_sb, tokTj_sb)
            )
            ps = psP.tile([128, 512], F32, tag="pj", name=f"pj{which}{pr}_{t}")
            for kc in range(0, NKC, 2):
                mm(
                    ps[:],
                    w_sb[:, pr, kc : kc + 2, :],
                    rhs_sb[:, t, kc : kc + 2, :],
                    start=(kc == 0),
                    stop=(kc == NKC - 2),
                    perf_mode=DR,
                )
            nc.vector.tensor_copy(out=dst[:, pr, t * 512 : (t + 1) * 512], in_=ps[:])

        def proj_v(ic, nb):
            """fp8 DoubleRow V projection for one i-chunk and one 8-head half
            via a borrowed scores-pool slot. PSUM is x256; scale by 0.25 into
            fp8 (std ~41, max ~5 sigma = 205 << 448: no clipping)."""
            for nb in (nb,):
                ps = psP.tile([128, 512], F32, tag="pj", name=f"pjv{ic}_{nb}")
                for kc in range(0, NKC, 2):
                    mm(
                        ps[:],
                        tokT_sb[:, ic // 4, kc : kc + 2,
                                (ic % 4) * 128 : (ic % 4 + 1) * 128],
                        wv_sb[:, nb, kc : kc + 2, :],
                        start=(kc == 0),
                        stop=(kc == NKC - 2),
                        perf_mode=DR,
                    )
                nc.vector.tensor_scalar_mul(
                    v_sb[:, ic, nb * 8 : (nb + 1) * 8, 0:DV],
                    ps.rearrange("p (h v) -> p h v", h=8),
                    VSCALE / WSCALE,
                )

        def normalize_stage1(pr, acc, sweep):
            """Pop-time half of multi^T = heads^T / rowsum: drain acc to SBUF
            (both heads' copies back-to-back, keeping the DVE queue clear of
            roundtrip-blocked ops) and launch the rowsum DRAM broadcast."""
            ctx = []
            for hh in range(2):
                h = 2 * pr + hh
                hraw = pn_pool.tile(
                    [DV + 1, 512], F32, tag="hraw", name=f"hraw{sweep}_{h}"
                )
                nc.vector.tensor_copy(out=hraw[:], in_=acc[hh][:])  # frees acc
                ctx.append(hraw)
            for hh in range(2):
                h = 2 * pr + hh
                rs_dram = pdram.tile([1, 512], F32, tag="rsd", name=f"rsd{sweep}_{h}")
                nc.sync.dma_start(out=rs_dram[:], in_=ctx[hh][DV : DV + 1, :])
                rec_in = pn_pool.tile([DV, 512], F32, tag="rin", name=f"rin{sweep}_{h}")
                nc.gpsimd.dma_start(out=rec_in[:], in_=rs_dram.to_broadcast((DV, 512)))
                ctx.append(rec_in)
            return ctx

        def normalize_stage2(pr, ctx, sweep):
            """Deferred half: recips run with their broadcast long landed, so
            they never block the in-order DVE queue."""
            j0 = sweep * 512
            for hh in range(2):
                hraw, rec_in = ctx[hh], ctx[2 + hh]
                nc.vector.reciprocal_approx_fast(out=rec_in[:], in_=rec_in[:])
                if hh == 0:
                    nc.gpsimd.tensor_tensor(
                        multiT[pr // 2][0:64, pr % 2, j0 : j0 + 512],
                        hraw[0:DV, :], rec_in[:], mult,
                    )
                else:
                    tmp64 = pn_pool.tile(
                        [DV, 512], F8, tag="tmp64", name=f"tmp{sweep}_{2 * pr + hh}"
                    )
                    nc.gpsimd.tensor_tensor(tmp64[:], hraw[0:DV, :], rec_in[:], mult)
                    nc.sync.dma_start(
                        out=multiT[pr // 2][64:128, pr % 2, j0 : j0 + 512],
                        in_=tmp64[:],
                    )

        def attention(hooks_by_sweep, after_pair=None):
            """Two j-half sweeps over all pairs. attnV runs fp8 DoubleRow over
            i-chunk PAIRS: e tiles are [128, 2, 512] fp8 (middle dim = the two
            i-chunks of the pair), V is fp8 at x64, so each attnV matmul
            contracts 256 i's in one 512-column pass. The exp stream is split:
            most tiles on the Scalar engine (Exp -> fp8 out), a slice on the
            DVE via the e4m3 bit-trick int8(x*EXP_A + EXP_B) whose saturation
            at -128 doubles as an underflow clamp (-0.0). hooks_by_sweep
            [(pr, sweep)][ic] is a list of thunks; pending pops run BEFORE
            each iteration's scores to fill the exp wait."""
            from collections import deque

            pending = deque()   # (eT_pair, m, pr, acc, sweep), newest at right
            norm_q = deque()    # (ready_gic, pr, ctx, sweep): deferred stage2
            gic = [0]           # global ic counter across all blocks
            exp_n = [0]         # global exp-tile counter for ACT/DVE split

            def flush_norms(limit_gic=None):
                while norm_q and (limit_gic is None or norm_q[0][0] <= limit_gic):
                    _, npr, nctx, nsweep = norm_q.popleft()
                    normalize_stage2(npr, nctx, nsweep)

            def do_attnv(peT, m, ppr, pacc, psweep):
                for hh in range(2):
                    mm(
                        pacc[hh][:],
                        v_sb[:, 2 * m : 2 * m + 2, 2 * ppr + hh, :],
                        peT[hh][:],
                        start=(m == 0),
                        stop=(m == NIC // 2 - 1),
                        perf_mode=DR,
                    )

            for pr in range(NPAIR):
                for sweep in range(2):
                    j0 = sweep * 512
                    acc = [
                        psAcc.tile(
                            [DV + 1, 512], F32, tag="acc", name=f"acc{sweep}_{pr}_{hh}"
                        )
                        for hh in range(2)
                    ]
                    hooks = hooks_by_sweep.get((pr, sweep), {})
                    # Deep attnV lag (in ic-pair units): an attnV pop trails
                    # its exps by ~6 pairs, so neither ACT/DVE exp latency nor
                    # queueing ever stalls the PE. psAcc's 2 buffers stay
                    # consistent: block B's pops drain during block B+1, whose
                    # own pops drain during B+2, steady-state.
                    limit = 7 if pr == 0 else 6
                    cur = None
                    for ic in range(NIC):
                        gic[0] += 1
                        flush_norms(gic[0])
                        for fn in hooks.get(ic, ()):
                            fn()
                        while len(pending) >= limit:
                            pa = pending.popleft()
                            do_attnv(*pa)
                            if pa[1] == NIC // 2 - 1:
                                ctx = normalize_stage1(pa[2], pa[3], pa[4])
                                norm_q.append((gic[0] + 3, pa[2], ctx, pa[4]))
                        # scores^T, row-tiled pair (K=64 at partitions 0/64);
                        # one 1-bank PSUM slot per (ic, head); the exp writes
                        # the matching plane of the DR-ready [128,2,512] tile.
                        # hh0 exps on ACT, hh1 on DVE for 3 of 4 pairs.
                        half = ic % 2
                        if half == 0:
                            cur = [
                                pe_pool.tile(
                                    [128, 2, 512], F8, tag="eT",
                                    name=f"eT{sweep}_{pr}_{ic // 2}_{hh}",
                                )
                                for hh in range(2)
                            ]
                            exp_n[0] += 1
                        dve_hh1 = exp_n[0] % DVE_PAIR_MOD < DVE_PAIR_CNT
                        for hh in range(2):
                            ps_s = psS.tile(
                                [128, 512], F32, tag="sc",
                                name=f"ps_s{sweep}_{pr}_{ic}_{hh}",
                            )
                            mm(
                                ps_s[:],
                                qT_sb[hh * 64 : (hh + 1) * 64, pr, ic * 128 : (ic + 1) * 128],
                                kT_sb[hh * 64 : (hh + 1) * 64, pr, j0 : j0 + 512],
                                start=True,
                                stop=True,
                            )
                            dst = cur[hh][:, half, :]
                            if hh == 1 and dve_hh1:
                                nc.vector.tensor_scalar(
                                    out=dst.bitcast(I8),
                                    in0=ps_s[:],
                                    scalar1=EXP_A,
                                    scalar2=EXP_B,
                                    op0=mult,
                                    op1=add,
                                )
                            else:
                                # x256-scaled q, k: 2^-16 inside EXP_SCALE
                                nc.scalar.activation(
                                    dst, ps_s[:], Exp, scale=EXP_SCALE
                                )
                        if half == 1:
                            pending.append((cur, ic // 2, pr, acc, sweep))
                    if after_pair and (sweep, pr) in after_pair:
                        after_pair[(sweep, pr)]()
            while pending:
                pa = pending.popleft()
                do_attnv(*pa)
                if pa[1] == NIC // 2 - 1:
                    ctx = normalize_stage1(pa[2], pa[3], pa[4])
                    norm_q.append((0, pa[2], ctx, pa[4]))
            flush_norms()

        # Hook schedule for pair-major order (pair p: j-half-0 block then
        # j-half-1 block). Each pair's j-half-1 block has no inherent hook
        # needs, so it carries the NEXT pair's q t0 / k chains and a share of
        # the deferred V projections — load stays near-uniform across all 256
        # iterations instead of crowding a single global sweep.
        hooks = {}

        def add_hook(pr, sweep, ic, fn):
            hooks.setdefault((pr, sweep), {}).setdefault(ic, []).append(fn)

        # V heads 0-7 (pairs 0-3): i-chunks 0-9 in pair 0's first block, the
        # lag-6 tail (i-chunks 10-15) in its second block just before each is
        # consumed. V heads 8-15 (first used at pair 4) spread over pairs
        # 1-3's second blocks.
        # one hook per iteration (chains own ics 3/5/7/11); every V keeps a
        # >=2-iteration lead on its lag-7 attnV pop (v_m pops at ic m+7)
        add_hook(0, 0, 1, lambda: proj_v(0, 0))
        add_hook(0, 0, 1, lambda: proj_v(1, 0))
        for ic, m in ((2, 2), (4, 3), (9, 4), (10, 5), (6, 6), (12, 7), (8, 8)):
            add_hook(0, 0, ic, lambda m=m: proj_v(m, 0))
        add_hook(0, 0, 14, lambda: proj_v(9, 0))
        add_hook(0, 0, 15, lambda: proj_v(10, 0))
        for k in range(5):
            add_hook(0, 1, k, lambda k=k: proj_v(11 + k, 0))
        for k, ic in enumerate((1, 3, 5, 13, 14, 15)):
            add_hook(1, 1, ic, lambda k=k: proj_v(k, 1))
        for k, ic in enumerate((1, 2, 3, 13, 14)):
            add_hook(2, 1, ic, lambda k=k: proj_v(6 + k, 1))
        for k, ic in enumerate((1, 2, 3)):
            add_hook(3, 1, ic, lambda k=k: proj_v(11 + k, 1))
        add_hook(4, 0, 12, lambda: proj_v(14, 1))
        add_hook(4, 0, 13, lambda: proj_v(15, 1))
        for pr in range(NPAIR):
            # own q t1-3, consumed by this pair's first block from ic4/8/12 on
            for t, ic in ((1, 3), (2, 7), (3, 11)):
                add_hook(pr, 0, ic, lambda pr=pr, t=t: proj_chain(pr, "q", t))
        add_hook(0, 0, 5, lambda: proj_chain(0, "k", 1))
        for pr in range(7):  # next pair's bootstrap rides the j-half-1 block
            add_hook(pr, 1, 7, lambda pr=pr: proj_chain(pr + 1, "q", 0))
            add_hook(pr, 1, 9, lambda pr=pr: proj_chain(pr + 1, "k", 0))
            add_hook(pr, 1, 11, lambda pr=pr: proj_chain(pr + 1, "k", 1))

        pc_tiles = {}

        def open_phase_c():
            # pa's tensors are all dead once sweep 0 ends; reuse the space for
            # phase C inputs so their DMA overlaps all of sweep 1.
            pa.release()
            pc = stack.enter_context(tc.tile_pool(name="pc", bufs=1))
            pc_tiles["wo"] = pc.tile([128, 2, NKC, 512], F8, name="wo_sb")
            pc_tiles["tokres"] = pc.tile([128, NJCH, D], F32, name="tokres_sb")
            nc.sync.dma_start(pc_tiles["wo"][:], wo_d[:])
            nc.sync.dma_start(pc_tiles["tokres"][:], tokres_d[:])

        # upfront: just enough projection for sweep-0 pair-0's first scores
        proj_chain(0, "q", 0)
        proj_chain(0, "k", 0)

        attention(hooks, after_pair={(0, 7): open_phase_c})
        wo_sb = pc_tiles["wo"]
        tokres_sb = pc_tiles["tokres"]
        paJ.release()
        psAcc.release()
        psP.release()
        psS.release()
        # ---------------- Phase C: out-proj + residual + LayerNorm ----------------
        with (
            tc.tile_pool(name="pC", bufs=4) as pC,
            tc.tile_pool(name="pStats", bufs=8) as pStats,
            tc.tile_pool(name="psC", bufs=4, space="PSUM") as psC,
        ):
            # Out-proj in two steps per jch: kc 0-6 accumulate early (their
            # multiT chunks are ready pairs before the last normalize), kc 7
            # finishes when multiT[7] lands. Prefilling 4 PSUM groups hides
            # the last normalize's DRAM round-trip behind ~12us of matmuls.
            prefill = {}

            def emit_prefill(jch):
                ps_o = psC.tile([128, D], F32, tag="po", name=f"ps_o{jch}")
                for kcp in range(NKC // 2 - 1):
                    lhsT = multiT[kcp][:, :, jch * 128 : (jch + 1) * 128]
                    for nb in range(2):
                        mm(
                            ps_o[:, nb * 512 : (nb + 1) * 512],
                            lhsT,
                            wo_sb[:, nb, 2 * kcp : 2 * kcp + 2, :],
                            start=(kcp == 0),
                            stop=False,
                            perf_mode=DR,
                        )
                prefill[jch] = ps_o

            for jch in range(4):
                emit_prefill(jch)
            for jch in range(NJCH):
                ps_o = prefill.pop(jch)
                lhsT = multiT[NKC // 2 - 1][:, :, jch * 128 : (jch + 1) * 128]
                for nb in range(2):
                    mm(
                        ps_o[:, nb * 512 : (nb + 1) * 512],
                        lhsT,
                        wo_sb[:, nb, NKC - 2 : NKC, :],
                        start=False,
                        stop=True,
                        perf_mode=DR,
                    )
                # x = psum + residual, sum_t = rowsum(x), in one DVE pass
                x_sb = pC.tile([128, D], F32, tag="x", name=f"x{jch}")
                sum_t = pStats.tile([128, 1], F32, tag="sum", name=f"sum{jch}")
                # x = psum/WSCALE + residual (undo the Wo fp8 pre-scale),
                # sum_t = rowsum(x), in one DVE pass
                nc.vector.scalar_tensor_tensor(
                    out=x_sb[:],
                    in0=ps_o[:],
                    scalar=1.0 / WSCALE,
                    in1=tokres_sb[:, jch, :],
                    op0=mult,
                    op1=add,
                    accum_out=sum_t[:],
                )
                negmean = pStats.tile([128, 1], F32, tag="nm", name=f"nm{jch}")
                nc.vector.tensor_scalar_mul(negmean[:], sum_t[:], -1.0 / D)
                # Variance always on Scalar (Square w/ accum); the final scale
                # alternates ACT/DVE by jch parity -- balances the two queues
                # at the drain with fewer cross-engine hops per chain.
                scrap = pC.tile([128, D], BF, tag="scrap", name=f"scrap{jch}")
                ssq = pStats.tile([128, 1], F32, tag="ssq", name=f"ssq{jch}")
                nc.scalar.activation(
                    scrap[:], x_sb[:], Square, bias=negmean[:], accum_out=ssq[:]
                )
                std_t = pStats.tile([128, 1], F32, tag="std", name=f"std{jch}")
                nc.scalar.activation(std_t[:], ssq[:], Sqrt, bias=eps_sb[:], scale=1.0 / D)
                rstd = pStats.tile([128, 1], F32, tag="rstd", name=f"rstd{jch}")
                nc.vector.reciprocal(rstd[:], std_t[:])
                # (x - m) * rstd == x*rstd + (negmean*rstd), one fused op
                rstd_nm = pStats.tile([128, 1], F32, tag="rnm", name=f"rnm{jch}")
                nc.vector.tensor_tensor(rstd_nm[:], negmean[:], rstd[:], mult)
                out_sb = pC.tile([128, D], F32, tag="out", name=f"out{jch}")
                if jch % 2 == 0:
                    nc.scalar.activation(
                        out_sb[:],
                        x_sb[:],
                        mybir.ActivationFunctionType.Identity,
                        bias=rstd_nm[:],
                        scale=rstd[:],
                    )
                else:
                    nc.vector.tensor_scalar(
                        out=out_sb[:], in0=x_sb[:], scalar1=rstd[:],
                        scalar2=rstd_nm[:], op0=mult, op1=add,
                    )
                if apply_affine:
                    nc.gpsimd.tensor_tensor(out_sb[:], out_sb[:], gamma_sb[:], mult)
                    nc.gpsimd.tensor_tensor(out_sb[:], out_sb[:], beta_sb[:], add)
                nc.sync.dma_start(out_d[:, jch], out_sb[:])
                # second prefill wave once the first four STTs are emitted, so
                # the PE chain never parks on a not-yet-freed PSUM group
                if jch == 3:
                    for j2 in range(4, NJCH):
                        emit_prefill(j2)

    nc.compile()
    return nc


def _prep_inputs(tokens, Wq, Wk, Wv, Wo, gamma, beta):
    """Host-side layout prep. Returns per-core input maps. All tensors use
    block-major layouts so every kernel DMA is contiguous per partition:
    [p, blk, kc, cols] with contraction row index kc*128+p."""
    tokens = np.ascontiguousarray(np.asarray(tokens, dtype=np.float32))

    def blocks(a, ncols):  # [1024, N] -> [128, N//ncols, NKC, ncols]
        return np.ascontiguousarray(
            a.reshape(NKC, 128, a.shape[-1] // ncols, ncols).transpose(1, 2, 0, 3)
        )

    wq_all = blocks(
        (np.asarray(Wq).transpose(1, 0, 2).reshape(D, H * DK) * WSCALE).astype(FP8),
        128,
    )
    wk_all = blocks(
        (np.asarray(Wk).transpose(1, 0, 2).reshape(D, H * DK) * WSCALE).astype(FP8),
        128,
    )
    wv_all = blocks(
        (np.asarray(Wv).transpose(1, 0, 2).reshape(D, H * DV) * WSCALE).astype(FP8),
        512,
    )
    wo_all = blocks((np.asarray(Wo) * WSCALE).astype(FP8), 512)
    gamma_bc = np.ascontiguousarray(
        np.broadcast_to(np.asarray(gamma, np.float32), (128, D))
    )
    beta_bc = np.ascontiguousarray(
        np.broadcast_to(np.asarray(beta, np.float32), (128, D))
    )

    tokT_by_b = []
    for b in range(B):
        tokT_by_b.append(blocks(tokens[b].T.astype(FP8), 512))  # [128,4,8,512]

    in_maps = []
    for c in range(NCORES):
        b, jc = c // 2, c % 2
        tokT = tokT_by_b[b]
        tokTj = np.ascontiguousarray(tokT[:, 2 * jc : 2 * jc + 2])
        tokres = np.ascontiguousarray(
            tokens[b, jc * JW : (jc + 1) * JW]
            .reshape(NJCH, 128, D)
            .transpose(1, 0, 2)
        )
        in_maps.append(
            {
                "tokT": tokT,
                "tokTj": tokTj,
                "wq": wq_all,
                "wk": wk_all,
                "wv": wv_all,
                "wo": wo_all,
                "tokres": tokres,
                "gamma_bc": gamma_bc,
                "beta_bc": beta_bc,
            }
        )
    return in_maps


def run(inputs, trace=False, tmpdir=None):
    """Run on hardware; returns (output, BassKernelResults)."""
    from concourse.bass_utils import run_bass_kernel_spmd

    apply_affine = not (
        np.all(np.asarray(inputs["gamma"]) == 1.0)
        and np.all(np.asarray(inputs["beta"]) == 0.0)
    )
    key = ("nc", apply_affine)
    if key not in _CACHE:
        _CACHE[key] = _build_nc(apply_affine)
    nc = _CACHE[key]
    in_maps = _prep_inputs(**inputs)
    res = run_bass_kernel_spmd(
        nc, in_maps, core_ids=list(range(NCORES)), trace=trace, tmpdir=tmpdir
    )
    out = np.empty((B, S, D), np.float32)
    for c in range(NCORES):
        b, jc = c // 2, c % 2
        o = res.results[c]["out"]  # [128, 8, 1024]
        out[b, jc * JW : (jc + 1) * JW] = (
            o.transpose(1, 0, 2).reshape(JW, D)
        )
    return out, res


def kernel(tokens, Wq, Wk, Wv, Wo, gamma, beta):
    out, _ = run(
        dict(tokens=tokens, Wq=Wq, Wk=Wk, Wv=Wv, Wo=Wo, gamma=gamma, beta=beta)
    )
    return out



# revision 90
# speedup vs baseline: 1.1023x; 1.1023x over previous
"""TRN2 Bass kernel for fused MHA (softmax-over-query quirk) + out-proj + residual + LayerNorm.

Problem shapes (hardcoded): tokens [4,2048,1024], Wq/Wk [16,1024,64], Wv [16,1024,64],
Wo [1024,1024], gamma/beta [1024]. Output [4,2048,1024] fp32.

Sharding: 8 cores, core c owns (batch b=c//2, S-half jc=c%2) of the OUTPUT rows.
No collectives. Each core computes, for its batch b:
  qT[dk,i] (full S), kT[dk,j] (its half) in bf16, V[i,dv] in fp8 (x64),
  scores^T[i,j] = q_i.k_j (PSUM fp32), e = exp(scores/8) in fp8e4m3,
  heads^T[dv,j] + rowsum row via a x64 ones-column appended to V,
  multi^T = heads^T / rowsum, out = multi @ Wo + tokens, LayerNorm rows.

All four projections AND the attention-value contraction run in fp8e4m3
DoubleRow (2 K-planes per matmul): tokens/multi cast to fp8, weights x256
(dodges e4m3 subnormals; the scales cancel exactly through softmax and the
out-proj's 2^8 is divided out in the residual-add STT). attnV pairs two
i-chunks per pass: each exp writes one plane of a DoubleRow-ready
[128, 2, 512] fp8 e-tile, V is stored [i, ic, h, dv|ones] so v_sb[:,2m:2m+2,h,:]
is the matching [128,2,65] stationary block -- attnV matmul count halves.

The exp stream (the former bottleneck) is split across engines: hh0 exps on
Scalar (Exp -> fp8 out, same cost as bf16), hh1 exps on the DVE for 3 of 4
ic-pairs via the e4m3 bit-trick int8(round(x*8*log2e + 56)) bitcast to fp8
-- one tensor_scalar op whose int8 write-saturation doubles as an underflow
clamp (bits -128 = -0.0). The mod-4 pattern keeps each block's last pair
all-Scalar so block-boundary deps never queue behind the DVE.

Schedule: pair-major blocks (pr, sweep) x 16 i-chunks; projections ride as
hooks in iteration slots (each pair's j-half-1 block carries the next
pair's q/k chains and V projections). attnV pops trail their exps by 6-7
ic-PAIRS (deep lag: exp latency/queueing never stalls the PE; psAcc's two
banks stay consistent with pops draining one block late). Normalize is
split: pop-time stage1 (acc->SBUF drain + rowsum DRAM-broadcast roundtrip),
stage2 (reciprocal + GpSimd mults) deferred 3 iterations so the in-order
DVE queue never parks on the roundtrip. Matmuls are NOT chained (the
scheduler fills stalls). DRAM layouts are block-major so every DMA is
contiguous per partition. Phase C: fp8-DR out-proj with 4+4 PSUM prefill
waves, residual+LN with variance on Scalar and the final scale alternating
Scalar/DVE. Measured ~325us on 8 cores (prior session 380us, stub 513us),
rel err 8.2e-4 (gate 2e-2).
"""

import numpy as np
import ml_dtypes

BF16 = ml_dtypes.bfloat16
FP8 = ml_dtypes.float8_e4m3

B, S, D, H, DK, DV = 4, 2048, 1024, 16, 64, 64
NCORES = 8
NPAIR = 8     # head pairs
NKC = 8       # D // 128 contraction chunks
NIC = 16      # S // 128 i-chunks
JW = 1024     # j columns per core (S/2)
NJCH = 8      # JW // 128
LN_EPS = 1e-5
WSCALE = 256.0  # fp8 weight pre-scale (power of 2)
VSCALE = 64.0   # on-chip V scale: psum(x256) * 0.25 -> fp8 with no clipping
EXP_SCALE = 0.125 / (WSCALE * WSCALE)  # true scores/8 per PSUM unit
LOG2E = 1.4426950408889634
# DVE exp bit-trick: int8(round(x*EXP_A + EXP_B)) viewed as e4m3 ~= exp(x*EXP_SCALE)
EXP_A = 8.0 * LOG2E * EXP_SCALE
EXP_B = 56.0  # e4m3 exponent bias 7 << 3 mantissa bits
DVE_PAIR_MOD = 4   # of every 4 ic-pairs, this many run their hh1 exp on the DVE
DVE_PAIR_CNT = 3   # (hh0 always on ACT: split pairs have latency max, not sum)

_CACHE = {}


def _build_nc(apply_affine):
    import concourse.tile as tile
    from concourse import bacc, mybir

    F32 = mybir.dt.float32
    BF = mybir.dt.bfloat16
    F8 = mybir.dt.float8e4
    I8 = mybir.dt.int8
    Exp = mybir.ActivationFunctionType.Exp
    Copy = mybir.ActivationFunctionType.Copy
    Square = mybir.ActivationFunctionType.Square
    Sqrt = mybir.ActivationFunctionType.Sqrt
    mult = mybir.AluOpType.mult
    add = mybir.AluOpType.add
    DR = mybir.MatmulPerfMode.DoubleRow

    nc = bacc.Bacc(
        "TRN2",
        target_bir_lowering=False,
        debug=False,
        enable_asserts=False,
        num_devices=NCORES,
    )

    # DRAM I/O (per-core views; host prepares layouts). Block-major orders
    # (i-blocks of 512 for tokens, head-pair blocks for Wq/Wk, head-half
    # blocks for Wv/Wo) make every DMA the kernel issues contiguous per
    # partition -- high descriptor efficiency, fast startup ramp.
    tokT_d = nc.dram_tensor("tokT", (128, 4, NKC, 512), F8, kind="ExternalInput").ap()
    tokTj_d = nc.dram_tensor("tokTj", (128, 2, NKC, 512), F8, kind="ExternalInput").ap()
    wq_d = nc.dram_tensor("wq", (128, NPAIR, NKC, 128), F8, kind="ExternalInput").ap()
    wk_d = nc.dram_tensor("wk", (128, NPAIR, NKC, 128), F8, kind="ExternalInput").ap()
    wv_d = nc.dram_tensor("wv", (128, 2, NKC, 512), F8, kind="ExternalInput").ap()
    wo_d = nc.dram_tensor("wo", (128, 2, NKC, 512), F8, kind="ExternalInput").ap()
    tokres_d = nc.dram_tensor("tokres", (128, NJCH, D), F32, kind="ExternalInput").ap()
    if apply_affine:
        gamma_d = nc.dram_tensor("gamma_bc", (128, D), F32, kind="ExternalInput").ap()
        beta_d = nc.dram_tensor("beta_bc", (128, D), F32, kind="ExternalInput").ap()
    out_d = nc.dram_tensor("out", (128, NJCH, D), F32, kind="ExternalOutput").ap()
    from contextlib import ExitStack

    from concourse.bass import _add_dep_helper

    # Chain all PE matmuls in emission order: stops the scheduler from
    # interleaving row-conflicting matmuls and keeps the stream dense.
    CHAIN_MM = False
    _prev_mm = [None]

    def mm(*args, **kwargs):
        inst = nc.tensor.matmul(*args, **kwargs)
        if CHAIN_MM and _prev_mm[0] is not None:
            _add_dep_helper(inst.ins, _prev_mm[0].ins, sync=False, reason="pe-order")
        _prev_mm[0] = inst
        return inst

    with tile.TileContext(nc) as tc, ExitStack() as stack:
        persist = stack.enter_context(tc.tile_pool(name="persist", bufs=1))
        qT_sb = persist.tile([128, NPAIR, S], BF)          # [pair-dk, pr, i]
        kT_sb = persist.tile([128, NPAIR, JW], BF)         # [pair-dk, pr, j]
        v_sb = persist.tile([128, NIC, H, DV + 1], F8)     # [i%128, ic, h, dv|64s]
        # multi^T in fp8, one tile per KC-PAIR ([128, 2, JW], middle dim = the
        # DoubleRow K-plane) so the out-proj runs fp8 DoubleRow; per-pair-ish
        # tiles keep out-proj dep tracking from serializing on the last write
        multiT = [
            persist.tile([128, 2, JW], F8, name=f"multiT{i}") for i in range(NKC // 2)
        ]
        eps_sb = persist.tile([128, 1], F32)
        if apply_affine:
            gamma_sb = persist.tile([128, D], F32)
            beta_sb = persist.tile([128, D], F32)
            nc.sync.dma_start(gamma_sb[:], gamma_d[:])
            nc.sync.dma_start(beta_sb[:], beta_d[:])
        nc.vector.memset(eps_sb[:], LN_EPS)
        for ic in range(NIC):
            # ones column scaled by VSCALE so rowsum matches the x64 V scale
            nc.vector.memset(v_sb[:, ic, :, DV : DV + 1], VSCALE)

        # pools that outlive pa must be allocated first (LIFO release)
        # psS: six 1-bank [128,512] slots shared by the scores ring and the
        # projection-chain borrows -- deep enough that neither a scores matmul
        # nor a proj chain ever waits on a slot whose exp hasn't fired yet.
        psS = tc.alloc_tile_pool(name="psS", bufs=6, space="PSUM")
        psAcc = tc.alloc_tile_pool(name="psAcc", bufs=2, space="PSUM")
        pe_pool = stack.enter_context(tc.tile_pool(name="pe", bufs=22))
        pn_pool = stack.enter_context(tc.tile_pool(name="pn", bufs=6))
        pdram = stack.enter_context(tc.tile_pool(name="pdram", bufs=6, space="DRAM"))

        # tokTj and wk outlive pa (the k j-half-1 chains consume them in
        # sweep 1), so they live in their own right-side pool.
        paJ = tc.alloc_tile_pool(name="paJ", bufs=1, side="right")
        tokTj_sb = paJ.tile([128, 2, NKC, 512], F8)
        wk_sb = paJ.tile([128, NPAIR, NKC, 128], F8)
        pa = tc.alloc_tile_pool(name="pa", bufs=1)
        wq_sb = pa.tile([128, NPAIR, NKC, 128], F8)
        tokT_sb = pa.tile([128, 4, NKC, 512], F8)
        wv_sb = pa.tile([128, 2, NKC, 512], F8)

        # Startup DMA over both HWDGE rings (SP + ACT, independent FIFOs).
        # Every transfer below is contiguous per partition. Critical path to
        # the first exp: wq pair-0 + tokens i-block-0 + wk pair-0 + tokTj
        # j-half-0; everything else queues behind.
        nc.sync.dma_start(wq_sb[:, 0], wq_d[:, 0])
        nc.scalar.dma_start(tokT_sb[:, 0], tokT_d[:, 0])
        nc.sync.dma_start(wk_sb[:, 0], wk_d[:, 0])
        nc.sync.dma_start(tokTj_sb[:, 0], tokTj_d[:, 0])
        nc.scalar.dma_start(wv_sb[:], wv_d[:])
        nc.sync.dma_start(tokTj_sb[:, 1], tokTj_d[:, 1])
        nc.scalar.dma_start(tokT_sb[:, 1], tokT_d[:, 1])
        nc.sync.dma_start(tokT_sb[:, 2], tokT_d[:, 2])
        nc.sync.dma_start(tokT_sb[:, 3], tokT_d[:, 3])
        nc.sync.dma_start(wq_sb[:, 1:], wq_d[:, 1:])
        nc.sync.dma_start(wk_sb[:, 1:], wk_d[:, 1:])

        def proj_chain(pr, which, t):
            """One 512-wide fp8 DoubleRow projection chain via a borrowed
            scores-pool slot."""
            w_sb, dst, rhs_sb = (
                (wq_sb, qT_sb, tokT_sb) if which == "q" else (wk_sb, kT_sb, tokTj_sb)
            )
            ps = psS.tile([128, 512], F32, tag="sc", name=f"pj{which}{pr}_{t}")
            for kc in range(0, NKC, 2):
                mm(
                    ps[:],
                    w_sb[:, pr, kc : kc + 2, :],
                    rhs_sb[:, t, kc : kc + 2, :],
                    start=(kc == 0),
                    stop=(kc == NKC - 2),
                    perf_mode=DR,
                )
            nc.vector.tensor_copy(out=dst[:, pr, t * 512 : (t + 1) * 512], in_=ps[:])

        def proj_v(ic, nb):
            """fp8 DoubleRow V projection for one i-chunk and one 8-head half
            via a borrowed scores-pool slot. PSUM is x256; scale by 0.25 into
            fp8 (std ~41, max ~5 sigma = 205 << 448: no clipping)."""
            for nb in (nb,):
                ps = psS.tile([128, 512], F32, tag="sc", name=f"pjv{ic}_{nb}")
                for kc in range(0, NKC, 2):
                    mm(
                        ps[:],
                        tokT_sb[:, ic // 4, kc : kc + 2,
                                (ic % 4) * 128 : (ic % 4 + 1) * 128],
                        wv_sb[:, nb, kc : kc + 2, :],
                        start=(kc == 0),
                        stop=(kc == NKC - 2),
                        perf_mode=DR,
                    )
                nc.vector.tensor_scalar_mul(
                    v_sb[:, ic, nb * 8 : (nb + 1) * 8, 0:DV],
                    ps.rearrange("p (h v) -> p h v", h=8),
                    VSCALE / WSCALE,
                )

        def normalize_stage1(pr, acc, sweep):
            """Pop-time half of multi^T = heads^T / rowsum: drain acc to SBUF
            (both heads' copies back-to-back, keeping the DVE queue clear of
            roundtrip-blocked ops) and launch the rowsum DRAM broadcast."""
            ctx = []
            for hh in range(2):
                h = 2 * pr + hh
                hraw = pn_pool.tile(
                    [DV + 1, 512], F32, tag="hraw", name=f"hraw{sweep}_{h}"
                )
                nc.vector.tensor_copy(out=hraw[:], in_=acc[hh][:])  # frees acc
                ctx.append(hraw)
            for hh in range(2):
                h = 2 * pr + hh
                rs_dram = pdram.tile([1, 512], F32, tag="rsd", name=f"rsd{sweep}_{h}")
                nc.sync.dma_start(out=rs_dram[:], in_=ctx[hh][DV : DV + 1, :])
                rec_in = pn_pool.tile([DV, 512], F32, tag="rin", name=f"rin{sweep}_{h}")
                nc.gpsimd.dma_start(out=rec_in[:], in_=rs_dram.to_broadcast((DV, 512)))
                ctx.append(rec_in)
            return ctx

        def normalize_stage2(pr, ctx, sweep):
            """Deferred half: recips run with their broadcast long landed, so
            they never block the in-order DVE queue."""
            j0 = sweep * 512
            for hh in range(2):
                hraw, rec_in = ctx[hh], ctx[2 + hh]
                nc.vector.reciprocal_approx_fast(out=rec_in[:], in_=rec_in[:])
                if hh == 0:
                    nc.gpsimd.tensor_tensor(
                        multiT[pr // 2][0:64, pr % 2, j0 : j0 + 512],
                        hraw[0:DV, :], rec_in[:], mult,
                    )
                else:
                    tmp64 = pn_pool.tile(
                        [DV, 512], F8, tag="tmp64", name=f"tmp{sweep}_{2 * pr + hh}"
                    )
                    nc.gpsimd.tensor_tensor(tmp64[:], hraw[0:DV, :], rec_in[:], mult)
                    nc.sync.dma_start(
                        out=multiT[pr // 2][64:128, pr % 2, j0 : j0 + 512],
                        in_=tmp64[:],
                    )

        def attention(hooks_by_sweep, after_pair=None):
            """Two j-half sweeps over all pairs. attnV runs fp8 DoubleRow over
            i-chunk PAIRS: e tiles are [128, 2, 512] fp8 (middle dim = the two
            i-chunks of the pair), V is fp8 at x64, so each attnV matmul
            contracts 256 i's in one 512-column pass. The exp stream is split:
            most tiles on the Scalar engine (Exp -> fp8 out), a slice on the
            DVE via the e4m3 bit-trick int8(x*EXP_A + EXP_B) whose saturation
            at -128 doubles as an underflow clamp (-0.0). hooks_by_sweep
            [(pr, sweep)][ic] is a list of thunks; pending pops run BEFORE
            each iteration's scores to fill the exp wait."""
            from collections import deque

            pending = deque()   # (eT_pair, m, pr, acc, sweep), newest at right
            norm_q = deque()    # (ready_gic, pr, ctx, sweep): deferred stage2
            gic = [0]           # global ic counter across all blocks
            exp_n = [0]         # global exp-tile counter for ACT/DVE split

            def flush_norms(limit_gic=None):
                while norm_q and (limit_gic is None or norm_q[0][0] <= limit_gic):
                    _, npr, nctx, nsweep = norm_q.popleft()
                    normalize_stage2(npr, nctx, nsweep)

            def do_attnv(peT, m, ppr, pacc, psweep):
                for hh in range(2):
                    mm(
                        pacc[hh][:],
                        v_sb[:, 2 * m : 2 * m + 2, 2 * ppr + hh, :],
                        peT[hh][:],
                        start=(m == 0),
                        stop=(m == NIC // 2 - 1),
                        perf_mode=DR,
                    )

            for pr in range(NPAIR):
                for sweep in range(2):
                    j0 = sweep * 512
                    acc = [
                        psAcc.tile(
                            [DV + 1, 512], F32, tag="acc", name=f"acc{sweep}_{pr}_{hh}"
                        )
                        for hh in range(2)
                    ]
                    hooks = hooks_by_sweep.get((pr, sweep), {})
                    # Deep attnV lag (in ic-pair units): an attnV pop trails
                    # its exps by ~6 pairs, so neither ACT/DVE exp latency nor
                    # queueing ever stalls the PE. psAcc's 2 buffers stay
                    # consistent: block B's pops drain during block B+1, whose
                    # own pops drain during B+2, steady-state.
                    limit = 7 if pr == 0 else 6
                    cur = None
                    for ic in range(NIC):
                        gic[0] += 1
                        flush_norms(gic[0])
                        for fn in hooks.get(ic, ()):
                            fn()
                        while len(pending) >= limit:
                            pa = pending.popleft()
                            do_attnv(*pa)
                            if pa[1] == NIC // 2 - 1:
                                ctx = normalize_stage1(pa[2], pa[3], pa[4])
                                norm_q.append((gic[0] + 3, pa[2], ctx, pa[4]))
                        # scores^T, row-tiled pair (K=64 at partitions 0/64);
                        # one 1-bank PSUM slot per (ic, head); the exp writes
                        # the matching plane of the DR-ready [128,2,512] tile.
                        # hh0 exps on ACT, hh1 on DVE for 3 of 4 pairs.
                        half = ic % 2
                        if half == 0:
                            cur = [
                                pe_pool.tile(
                                    [128, 2, 512], F8, tag="eT",
                                    name=f"eT{sweep}_{pr}_{ic // 2}_{hh}",
                                )
                                for hh in range(2)
                            ]
                            exp_n[0] += 1
                        dve_hh1 = exp_n[0] % DVE_PAIR_MOD < DVE_PAIR_CNT
                        for hh in range(2):
                            ps_s = psS.tile(
                                [128, 512], F32, tag="sc",
                                name=f"ps_s{sweep}_{pr}_{ic}_{hh}",
                            )
                            mm(
                                ps_s[:],
                                qT_sb[hh * 64 : (hh + 1) * 64, pr, ic * 128 : (ic + 1) * 128],
                                kT_sb[hh * 64 : (hh + 1) * 64, pr, j0 : j0 + 512],
                                start=True,
                                stop=True,
                            )
                            dst = cur[hh][:, half, :]
                            if hh == 1 and dve_hh1:
                                nc.vector.tensor_scalar(
                                    out=dst.bitcast(I8),
                                    in0=ps_s[:],
                                    scalar1=EXP_A,
                                    scalar2=EXP_B,
                                    op0=mult,
                                    op1=add,
                                )
                            else:
                                # x256-scaled q, k: 2^-16 inside EXP_SCALE
                                nc.scalar.activation(
                                    dst, ps_s[:], Exp, scale=EXP_SCALE
                                )
                        if half == 1:
                            pending.append((cur, ic // 2, pr, acc, sweep))
                    if after_pair and (sweep, pr) in after_pair:
                        after_pair[(sweep, pr)]()
            while pending:
                pa = pending.popleft()
                do_attnv(*pa)
                if pa[1] == NIC // 2 - 1:
                    ctx = normalize_stage1(pa[2], pa[3], pa[4])
                    norm_q.append((0, pa[2], ctx, pa[4]))
            flush_norms()

        # Hook schedule for pair-major order (pair p: j-half-0 block then
        # j-half-1 block). Each pair's j-half-1 block has no inherent hook
        # needs, so it carries the NEXT pair's q t0 / k chains and a share of
        # the deferred V projections — load stays near-uniform across all 256
        # iterations instead of crowding a single global sweep.
        hooks = {}

        def add_hook(pr, sweep, ic, fn):
            hooks.setdefault((pr, sweep), {}).setdefault(ic, []).append(fn)

        # V heads 0-7 (pairs 0-3): i-chunks 0-9 in pair 0's first block, the
        # lag-6 tail (i-chunks 10-15) in its second block just before each is
        # consumed. V heads 8-15 (first used at pair 4) spread over pairs
        # 1-3's second blocks.
        # one hook per iteration (chains own ics 3/5/7/11); every V keeps a
        # >=2-iteration lead on its lag-7 attnV pop (v_m pops at ic m+7)
        add_hook(0, 0, 1, lambda: proj_v(0, 0))
        add_hook(0, 0, 1, lambda: proj_v(1, 0))
        for ic, m in ((2, 2), (4, 3), (9, 4), (10, 5), (6, 6), (12, 7), (8, 8)):
            add_hook(0, 0, ic, lambda m=m: proj_v(m, 0))
        add_hook(0, 0, 14, lambda: proj_v(9, 0))
        add_hook(0, 0, 15, lambda: proj_v(10, 0))
        for k in range(5):
            add_hook(0, 1, k, lambda k=k: proj_v(11 + k, 0))
        for k, ic in enumerate((1, 3, 5, 13, 14, 15)):
            add_hook(1, 1, ic, lambda k=k: proj_v(k, 1))
        for k, ic in enumerate((1, 2, 3, 13, 14)):
            add_hook(2, 1, ic, lambda k=k: proj_v(6 + k, 1))
        for k, ic in enumerate((1, 2, 3)):
            add_hook(3, 1, ic, lambda k=k: proj_v(11 + k, 1))
        add_hook(4, 0, 12, lambda: proj_v(14, 1))
        add_hook(4, 0, 13, lambda: proj_v(15, 1))
        for pr in range(NPAIR):
            # own q t1-3, consumed by this pair's first block from ic4/8/12 on
            for t, ic in ((1, 3), (2, 7), (3, 11)):
                add_hook(pr, 0, ic, lambda pr=pr, t=t: proj_chain(pr, "q", t))
        add_hook(0, 0, 5, lambda: proj_chain(0, "k", 1))
        for pr in range(7):  # next pair's bootstrap rides the j-half-1 block
            add_hook(pr, 1, 7, lambda pr=pr: proj_chain(pr + 1, "q", 0))
            add_hook(pr, 1, 9, lambda pr=pr: proj_chain(pr + 1, "k", 0))
            add_hook(pr, 1, 11, lambda pr=pr: proj_chain(pr + 1, "k", 1))

        pc_tiles = {}

        def open_phase_c():
            # pa's tensors are all dead once sweep 0 ends; reuse the space for
            # phase C inputs so their DMA overlaps all of sweep 1.
            pa.release()
            pc = stack.enter_context(tc.tile_pool(name="pc", bufs=1))
            pc_tiles["wo"] = pc.tile([128, 2, NKC, 512], F8, name="wo_sb")
            pc_tiles["tokres"] = pc.tile([128, NJCH, D], F32, name="tokres_sb")
            nc.sync.dma_start(pc_tiles["wo"][:], wo_d[:])
            nc.sync.dma_start(pc_tiles["tokres"][:], tokres_d[:])

        # upfront: just enough projection for sweep-0 pair-0's first scores
        proj_chain(0, "q", 0)
        proj_chain(0, "k", 0)

        attention(hooks, after_pair={(0, 7): open_phase_c})
        wo_sb = pc_tiles["wo"]
        tokres_sb = pc_tiles["tokres"]
        paJ.release()
        psAcc.release()
        psS.release()
        # ---------------- Phase C: out-proj + residual + LayerNorm ----------------
        with (
            tc.tile_pool(name="pC", bufs=4) as pC,
            tc.tile_pool(name="pStats", bufs=8) as pStats,
            tc.tile_pool(name="psC", bufs=4, space="PSUM") as psC,
        ):
            # Out-proj in two steps per jch: kc 0-6 accumulate early (their
            # multiT chunks are ready pairs before the last normalize), kc 7
            # finishes when multiT[7] lands. Prefilling 4 PSUM groups hides
            # the last normalize's DRAM round-trip behind ~12us of matmuls.
            prefill = {}

            def emit_prefill(jch):
                ps_o = psC.tile([128, D], F32, tag="po", name=f"ps_o{jch}")
                for kcp in range(NKC // 2 - 1):
                    lhsT = multiT[kcp][:, :, jch * 128 : (jch + 1) * 128]
                    for nb in range(2):
                        mm(
                            ps_o[:, nb * 512 : (nb + 1) * 512],
                            lhsT,
                            wo_sb[:, nb, 2 * kcp : 2 * kcp + 2, :],
                            start=(kcp == 0),
                            stop=False,
                            perf_mode=DR,
                        )
                prefill[jch] = ps_o

            for jch in range(4):
                emit_prefill(jch)
            for jch in range(NJCH):
                ps_o = prefill.pop(jch)
                lhsT = multiT[NKC // 2 - 1][:, :, jch * 128 : (jch + 1) * 128]
                for nb in range(2):
                    mm(
                        ps_o[:, nb * 512 : (nb + 1) * 512],
                        lhsT,
                        wo_sb[:, nb, NKC - 2 : NKC, :],
                        start=False,
                        stop=True,
                        perf_mode=DR,
                    )
                # x = psum + residual, sum_t = rowsum(x), in one DVE pass
                x_sb = pC.tile([128, D], F32, tag="x", name=f"x{jch}")
                sum_t = pStats.tile([128, 1], F32, tag="sum", name=f"sum{jch}")
                # x = psum/WSCALE + residual (undo the Wo fp8 pre-scale),
                # sum_t = rowsum(x), in one DVE pass
                nc.vector.scalar_tensor_tensor(
                    out=x_sb[:],
                    in0=ps_o[:],
                    scalar=1.0 / WSCALE,
                    in1=tokres_sb[:, jch, :],
                    op0=mult,
                    op1=add,
                    accum_out=sum_t[:],
                )
                negmean = pStats.tile([128, 1], F32, tag="nm", name=f"nm{jch}")
                nc.vector.tensor_scalar_mul(negmean[:], sum_t[:], -1.0 / D)
                # Variance always on Scalar (Square w/ accum); the final scale
                # alternates ACT/DVE by jch parity -- balances the two queues
                # at the drain with fewer cross-engine hops per chain.
                scrap = pC.tile([128, D], BF, tag="scrap", name=f"scrap{jch}")
                ssq = pStats.tile([128, 1], F32, tag="ssq", name=f"ssq{jch}")
                nc.scalar.activation(
                    scrap[:], x_sb[:], Square, bias=negmean[:], accum_out=ssq[:]
                )
                std_t = pStats.tile([128, 1], F32, tag="std", name=f"std{jch}")
                nc.scalar.activation(std_t[:], ssq[:], Sqrt, bias=eps_sb[:], scale=1.0 / D)
                rstd = pStats.tile([128, 1], F32, tag="rstd", name=f"rstd{jch}")
                nc.vector.reciprocal(rstd[:], std_t[:])
                # (x - m) * rstd == x*rstd + (negmean*rstd), one fused op
                rstd_nm = pStats.tile([128, 1], F32, tag="rnm", name=f"rnm{jch}")
                nc.vector.tensor_tensor(rstd_nm[:], negmean[:], rstd[:], mult)
                out_sb = pC.tile([128, D], F32, tag="out", name=f"out{jch}")
                if jch % 3 == 0:
                    nc.scalar.activation(
                        out_sb[:],
                        x_sb[:],
                        mybir.ActivationFunctionType.Identity,
                        bias=rstd_nm[:],
                        scale=rstd[:],
                    )
                elif jch % 3 == 1:
                    nc.vector.tensor_scalar(
                        out=out_sb[:], in0=x_sb[:], scalar1=rstd[:],
                        scalar2=rstd_nm[:], op0=mult, op1=add,
                    )
                else:
                    # third lane: GpSimd is idle at the drain (SBUF-only op)
                    nc.gpsimd.tensor_scalar(
                        out=out_sb[:], in0=x_sb[:], scalar1=rstd[:],
                        scalar2=rstd_nm[:], op0=mult, op1=add,
                    )
                if apply_affine:
                    nc.gpsimd.tensor_tensor(out_sb[:], out_sb[:], gamma_sb[:], mult)
                    nc.gpsimd.tensor_tensor(out_sb[:], out_sb[:], beta_sb[:], add)
                nc.sync.dma_start(out_d[:, jch], out_sb[:])
                # second prefill wave once the first four STTs are emitted, so
                # the PE chain never parks on a not-yet-freed PSUM group
                if jch == 3:
                    for j2 in range(4, NJCH):
                        emit_prefill(j2)

    nc.compile()
    return nc


def _prep_inputs(tokens, Wq, Wk, Wv, Wo, gamma, beta):
    """Host-side layout prep. Returns per-core input maps. All tensors use
    block-major layouts so every kernel DMA is contiguous per partition:
    [p, blk, kc, cols] with contraction row index kc*128+p."""
    tokens = np.ascontiguousarray(np.asarray(tokens, dtype=np.float32))

    def blocks(a, ncols):  # [1024, N] -> [128, N//ncols, NKC, ncols]
        return np.ascontiguousarray(
            a.reshape(NKC, 128, a.shape[-1] // ncols, ncols).transpose(1, 2, 0, 3)
        )

    wq_all = blocks(
        (np.asarray(Wq).transpose(1, 0, 2).reshape(D, H * DK) * WSCALE).astype(FP8),
        128,
    )
    wk_all = blocks(
        (np.asarray(Wk).transpose(1, 0, 2).reshape(D, H * DK) * WSCALE).astype(FP8),
        128,
    )
    wv_all = blocks(
        (np.asarray(Wv).transpose(1, 0, 2).reshape(D, H * DV) * WSCALE).astype(FP8),
        512,
    )
    wo_all = blocks((np.asarray(Wo) * WSCALE).astype(FP8), 512)
    gamma_bc = np.ascontiguousarray(
        np.broadcast_to(np.asarray(gamma, np.float32), (128, D))
    )
    beta_bc = np.ascontiguousarray(
        np.broadcast_to(np.asarray(beta, np.float32), (128, D))
    )

    tokT_by_b = []
    for b in range(B):
        tokT_by_b.append(blocks(tokens[b].T.astype(FP8), 512))  # [128,4,8,512]

    in_maps = []
    for c in range(NCORES):
        b, jc = c // 2, c % 2
        tokT = tokT_by_b[b]
        tokTj = np.ascontiguousarray(tokT[:, 2 * jc : 2 * jc + 2])
        tokres = np.ascontiguousarray(
            tokens[b, jc * JW : (jc + 1) * JW]
            .reshape(NJCH, 128, D)
            .transpose(1, 0, 2)
        )
        in_maps.append(
            {
                "tokT": tokT,
                "tokTj": tokTj,
                "wq": wq_all,
                "wk": wk_all,
                "wv": wv_all,
                "wo": wo_all,
                "tokres": tokres,
                "gamma_bc": gamma_bc,
                "beta_bc": beta_bc,
            }
        )
    return in_maps


def run(inputs, trace=False, tmpdir=None):
    """Run on hardware; returns (output, BassKernelResults)."""
    from concourse.bass_utils import run_bass_kernel_spmd

    apply_affine = not (
        np.all(np.asarray(inputs["gamma"]) == 1.0)
        and np.all(np.asarray(inputs["beta"]) == 0.0)
    )
    key = ("nc", apply_affine)
    if key not in _CACHE:
        _CACHE[key] = _build_nc(apply_affine)
    nc = _CACHE[key]
    in_maps = _prep_inputs(**inputs)
    res = run_bass_kernel_spmd(
        nc, in_maps, core_ids=list(range(NCORES)), trace=trace, tmpdir=tmpdir
    )
    out = np.empty((B, S, D), np.float32)
    for c in range(NCORES):
        b, jc = c // 2, c % 2
        o = res.results[c]["out"]  # [128, 8, 1024]
        out[b, jc * JW : (jc + 1) * JW] = (
            o.transpose(1, 0, 2).reshape(JW, D)
        )
    return out, res


def kernel(tokens, Wq, Wk, Wv, Wo, gamma, beta):
    out, _ = run(
        dict(tokens=tokens, Wq=Wq, Wk=Wk, Wv=Wv, Wo=Wo, gamma=gamma, beta=beta)
    )
    return out



# revision 91
# speedup vs baseline: 1.3073x; 1.1860x over previous
"""TRN2 Bass kernel for fused MHA (softmax-over-query quirk) + out-proj + residual + LayerNorm.

Problem shapes (hardcoded): tokens [4,2048,1024], Wq/Wk [16,1024,64], Wv [16,1024,64],
Wo [1024,1024], gamma/beta [1024]. Output [4,2048,1024] fp32.

Sharding: 8 cores, core c owns (batch b=c//2, S-half jc=c%2) of the OUTPUT rows.
No collectives. Each core computes, for its batch b:
  qT[dk,i] (full S), kT[dk,j] (its half) in bf16, V[i,dv] in fp8 (x64),
  scores^T[i,j] = q_i.k_j (PSUM fp32), e = exp(scores/8) in fp8e4m3,
  heads^T[dv,j] + rowsum row via a x64 ones-column appended to V,
  multi^T = heads^T / rowsum, out = multi @ Wo + tokens, LayerNorm rows.

All four projections AND the attention-value contraction run in fp8e4m3
DoubleRow (2 K-planes per matmul): tokens/multi cast to fp8, weights x256
(dodges e4m3 subnormals; the scales cancel exactly through softmax and the
out-proj's 2^8 is divided out in the residual-add STT). attnV pairs two
i-chunks per pass: each exp writes one plane of a DoubleRow-ready
[128, 2, 512] fp8 e-tile, V is stored [i, ic, h, dv|ones] so v_sb[:,2m:2m+2,h,:]
is the matching [128,2,65] stationary block -- attnV matmul count halves.

The exp stream (the former bottleneck) is split across engines: hh0 exps on
Scalar (Exp -> fp8 out, same cost as bf16), hh1 exps on the DVE for 3 of 4
ic-pairs via the e4m3 bit-trick int8(round(x*8*log2e + 56)) bitcast to fp8
-- one tensor_scalar op whose int8 write-saturation doubles as an underflow
clamp (bits -128 = -0.0). The mod-4 pattern keeps each block's last pair
all-Scalar so block-boundary deps never queue behind the DVE.

Schedule: pair-major blocks (pr, sweep) x 16 i-chunks; projections ride as
hooks in iteration slots (each pair's j-half-1 block carries the next
pair's q/k chains and V projections). attnV pops trail their exps by 6-7
ic-PAIRS (deep lag: exp latency/queueing never stalls the PE; psAcc's two
banks stay consistent with pops draining one block late). Normalize is
split: pop-time stage1 (acc->SBUF drain + rowsum DRAM-broadcast roundtrip),
stage2 (reciprocal + GpSimd mults) deferred 3 iterations so the in-order
DVE queue never parks on the roundtrip. Matmuls are NOT chained (the
scheduler fills stalls). DRAM layouts are block-major so every DMA is
contiguous per partition. Phase C: fp8-DR out-proj with 4+4 PSUM prefill
waves, residual+LN with variance on Scalar and the final scale alternating
Scalar/DVE. Measured ~325us on 8 cores (prior session 380us, stub 513us),
rel err 8.2e-4 (gate 2e-2).
"""

import numpy as np
import ml_dtypes

BF16 = ml_dtypes.bfloat16
FP8 = ml_dtypes.float8_e4m3

B, S, D, H, DK, DV = 4, 2048, 1024, 16, 64, 64
NCORES = 8
NPAIR = 8     # head pairs
NKC = 8       # D // 128 contraction chunks
NIC = 16      # S // 128 i-chunks
JW = 1024     # j columns per core (S/2)
NJCH = 8      # JW // 128
LN_EPS = 1e-5
WSCALE = 256.0  # fp8 weight pre-scale (power of 2)
VSCALE = 64.0   # on-chip V scale: psum(x256) * 0.25 -> fp8 with no clipping
EXP_SCALE = 0.125 / (WSCALE * WSCALE)  # true scores/8 per PSUM unit
LOG2E = 1.4426950408889634
# DVE exp bit-trick: int8(round(x*EXP_A + EXP_B)) viewed as e4m3 ~= exp(x*EXP_SCALE)
EXP_A = 8.0 * LOG2E * EXP_SCALE
EXP_B = 56.0  # e4m3 exponent bias 7 << 3 mantissa bits
DVE_PAIR_MOD = 4   # of every 4 ic-pairs, this many run their hh1 exp on the DVE
DVE_PAIR_CNT = 3   # (hh0 always on ACT: split pairs have latency max, not sum)

_CACHE = {}


def _build_nc(apply_affine):
    import concourse.tile as tile
    from concourse import bacc, mybir

    F32 = mybir.dt.float32
    BF = mybir.dt.bfloat16
    F8 = mybir.dt.float8e4
    I8 = mybir.dt.int8
    Exp = mybir.ActivationFunctionType.Exp
    Copy = mybir.ActivationFunctionType.Copy
    Square = mybir.ActivationFunctionType.Square
    Sqrt = mybir.ActivationFunctionType.Sqrt
    mult = mybir.AluOpType.mult
    add = mybir.AluOpType.add
    DR = mybir.MatmulPerfMode.DoubleRow

    nc = bacc.Bacc(
        "TRN2",
        target_bir_lowering=False,
        debug=False,
        enable_asserts=False,
        num_devices=NCORES,
    )

    # DRAM I/O (per-core views; host prepares layouts). Block-major orders
    # (i-blocks of 512 for tokens, head-pair blocks for Wq/Wk, head-half
    # blocks for Wv/Wo) make every DMA the kernel issues contiguous per
    # partition -- high descriptor efficiency, fast startup ramp.
    tokT_d = nc.dram_tensor("tokT", (128, 4, NKC, 512), F8, kind="ExternalInput").ap()
    tokTj_d = nc.dram_tensor("tokTj", (128, 2, NKC, 512), F8, kind="ExternalInput").ap()
    wq_d = nc.dram_tensor("wq", (128, NPAIR, NKC, 128), F8, kind="ExternalInput").ap()
    wk_d = nc.dram_tensor("wk", (128, NPAIR, NKC, 128), F8, kind="ExternalInput").ap()
    wv_d = nc.dram_tensor("wv", (128, 2, NKC, 512), F8, kind="ExternalInput").ap()
    wo_d = nc.dram_tensor("wo", (128, 2, NKC, 512), F8, kind="ExternalInput").ap()
    tokres_d = nc.dram_tensor("tokres", (128, NJCH, D), F32, kind="ExternalInput").ap()
    if apply_affine:
        gamma_d = nc.dram_tensor("gamma_bc", (128, D), F32, kind="ExternalInput").ap()
        beta_d = nc.dram_tensor("beta_bc", (128, D), F32, kind="ExternalInput").ap()
    out_d = nc.dram_tensor("out", (128, NJCH, D), F32, kind="ExternalOutput").ap()
    from contextlib import ExitStack

    from concourse.bass import _add_dep_helper

    # Chain all PE matmuls in emission order: stops the scheduler from
    # interleaving row-conflicting matmuls and keeps the stream dense.
    CHAIN_MM = False
    _prev_mm = [None]

    def mm(*args, **kwargs):
        inst = nc.tensor.matmul(*args, **kwargs)
        if CHAIN_MM and _prev_mm[0] is not None:
            _add_dep_helper(inst.ins, _prev_mm[0].ins, sync=False, reason="pe-order")
        _prev_mm[0] = inst
        return inst

    with tile.TileContext(nc) as tc, ExitStack() as stack:
        persist = stack.enter_context(tc.tile_pool(name="persist", bufs=1))
        qT_sb = persist.tile([128, NPAIR, S], BF)          # [pair-dk, pr, i]
        kT_sb = persist.tile([128, NPAIR, JW], BF)         # [pair-dk, pr, j]
        v_sb = persist.tile([128, NIC, H, DV + 1], F8)     # [i%128, ic, h, dv|64s]
        # multi^T in fp8, one tile per KC-PAIR ([128, 2, JW], middle dim = the
        # DoubleRow K-plane) so the out-proj runs fp8 DoubleRow; per-pair-ish
        # tiles keep out-proj dep tracking from serializing on the last write
        multiT = [
            persist.tile([128, 2, JW], F8, name=f"multiT{i}") for i in range(NKC // 2)
        ]
        eps_sb = persist.tile([128, 1], F32)
        if apply_affine:
            gamma_sb = persist.tile([128, D], F32)
            beta_sb = persist.tile([128, D], F32)
            nc.sync.dma_start(gamma_sb[:], gamma_d[:])
            nc.sync.dma_start(beta_sb[:], beta_d[:])
        nc.vector.memset(eps_sb[:], LN_EPS)
        for ic in range(NIC):
            # ones column scaled by VSCALE so rowsum matches the x64 V scale
            nc.vector.memset(v_sb[:, ic, :, DV : DV + 1], VSCALE)

        # pools that outlive pa must be allocated first (LIFO release)
        # psS: six 1-bank [128,512] slots shared by the scores ring and the
        # projection-chain borrows -- deep enough that neither a scores matmul
        # nor a proj chain ever waits on a slot whose exp hasn't fired yet.
        psS = tc.alloc_tile_pool(name="psS", bufs=6, space="PSUM")
        psAcc = tc.alloc_tile_pool(name="psAcc", bufs=2, space="PSUM")
        pe_pool = stack.enter_context(tc.tile_pool(name="pe", bufs=22))
        pn_pool = stack.enter_context(tc.tile_pool(name="pn", bufs=6))
        pdram = stack.enter_context(tc.tile_pool(name="pdram", bufs=6, space="DRAM"))

        # tokTj and wk outlive pa (the k j-half-1 chains consume them in
        # sweep 1), so they live in their own right-side pool.
        paJ = tc.alloc_tile_pool(name="paJ", bufs=1, side="right")
        tokTj_sb = paJ.tile([128, 2, NKC, 512], F8)
        wk_sb = paJ.tile([128, NPAIR, NKC, 128], F8)
        pa = tc.alloc_tile_pool(name="pa", bufs=1)
        wq_sb = pa.tile([128, NPAIR, NKC, 128], F8)
        tokT_sb = pa.tile([128, 4, NKC, 512], F8)
        wv_sb = pa.tile([128, 2, NKC, 512], F8)

        # Startup DMA over both HWDGE rings (SP + ACT, independent FIFOs).
        # Every transfer below is contiguous per partition. Critical path to
        # the first exp: wq pair-0 + tokens i-block-0 + wk pair-0 + tokTj
        # j-half-0; everything else queues behind.
        nc.sync.dma_start(wq_sb[:, 0], wq_d[:, 0])
        nc.scalar.dma_start(tokT_sb[:, 0], tokT_d[:, 0])
        nc.sync.dma_start(wk_sb[:, 0], wk_d[:, 0])
        nc.sync.dma_start(tokTj_sb[:, 0], tokTj_d[:, 0])
        nc.scalar.dma_start(wv_sb[:], wv_d[:])
        nc.sync.dma_start(tokTj_sb[:, 1], tokTj_d[:, 1])
        nc.scalar.dma_start(tokT_sb[:, 1], tokT_d[:, 1])
        nc.sync.dma_start(tokT_sb[:, 2], tokT_d[:, 2])
        nc.sync.dma_start(tokT_sb[:, 3], tokT_d[:, 3])
        nc.sync.dma_start(wq_sb[:, 1:], wq_d[:, 1:])
        nc.sync.dma_start(wk_sb[:, 1:], wk_d[:, 1:])

        def proj_chain(pr, which, t):
            """One 512-wide fp8 DoubleRow projection chain via a borrowed
            scores-pool slot."""
            w_sb, dst, rhs_sb = (
                (wq_sb, qT_sb, tokT_sb) if which == "q" else (wk_sb, kT_sb, tokTj_sb)
            )
            ps = psS.tile([128, 512], F32, tag="sc", name=f"pj{which}{pr}_{t}")
            for kc in range(0, NKC, 2):
                mm(
                    ps[:],
                    w_sb[:, pr, kc : kc + 2, :],
                    rhs_sb[:, t, kc : kc + 2, :],
                    start=(kc == 0),
                    stop=(kc == NKC - 2),
                    perf_mode=DR,
                )
            nc.vector.tensor_copy(out=dst[:, pr, t * 512 : (t + 1) * 512], in_=ps[:])

        def proj_v(ic, nb):
            """fp8 DoubleRow V projection for one i-chunk and one 8-head half
            via a borrowed scores-pool slot. PSUM is x256; scale by 0.25 into
            fp8 (std ~41, max ~5 sigma = 205 << 448: no clipping)."""
            for nb in (nb,):
                ps = psS.tile([128, 512], F32, tag="sc", name=f"pjv{ic}_{nb}")
                for kc in range(0, NKC, 2):
                    mm(
                        ps[:],
                        tokT_sb[:, ic // 4, kc : kc + 2,
                                (ic % 4) * 128 : (ic % 4 + 1) * 128],
                        wv_sb[:, nb, kc : kc + 2, :],
                        start=(kc == 0),
                        stop=(kc == NKC - 2),
                        perf_mode=DR,
                    )
                nc.vector.tensor_scalar_mul(
                    v_sb[:, ic, nb * 8 : (nb + 1) * 8, 0:DV],
                    ps.rearrange("p (h v) -> p h v", h=8),
                    VSCALE / WSCALE,
                )

        def normalize_stage1(pr, acc, sweep):
            """Pop-time half of multi^T = heads^T / rowsum: drain acc to SBUF
            (both heads' copies back-to-back, keeping the DVE queue clear of
            roundtrip-blocked ops) and launch the rowsum DRAM broadcast."""
            ctx = []
            for hh in range(2):
                h = 2 * pr + hh
                hraw = pn_pool.tile(
                    [DV + 1, 512], F32, tag="hraw", name=f"hraw{sweep}_{h}"
                )
                nc.vector.tensor_copy(out=hraw[:], in_=acc[hh][:])  # frees acc
                ctx.append(hraw)
            for hh in range(2):
                h = 2 * pr + hh
                rs_dram = pdram.tile([1, 512], F32, tag="rsd", name=f"rsd{sweep}_{h}")
                nc.sync.dma_start(out=rs_dram[:], in_=ctx[hh][DV : DV + 1, :])
                rec_in = pn_pool.tile([DV, 512], F32, tag="rin", name=f"rin{sweep}_{h}")
                nc.gpsimd.dma_start(out=rec_in[:], in_=rs_dram.to_broadcast((DV, 512)))
                ctx.append(rec_in)
            return ctx

        def normalize_stage2(pr, ctx, sweep):
            """Deferred half: recips run with their broadcast long landed, so
            they never block the in-order DVE queue."""
            j0 = sweep * 512
            for hh in range(2):
                hraw, rec_in = ctx[hh], ctx[2 + hh]
                nc.vector.reciprocal_approx_fast(out=rec_in[:], in_=rec_in[:])
                if hh == 0:
                    nc.gpsimd.tensor_tensor(
                        multiT[pr // 2][0:64, pr % 2, j0 : j0 + 512],
                        hraw[0:DV, :], rec_in[:], mult,
                    )
                else:
                    tmp64 = pn_pool.tile(
                        [DV, 512], F8, tag="tmp64", name=f"tmp{sweep}_{2 * pr + hh}"
                    )
                    nc.gpsimd.tensor_tensor(tmp64[:], hraw[0:DV, :], rec_in[:], mult)
                    nc.sync.dma_start(
                        out=multiT[pr // 2][64:128, pr % 2, j0 : j0 + 512],
                        in_=tmp64[:],
                    )

        def attention(hooks_by_sweep, after_pair=None):
            """Two j-half sweeps over all pairs. attnV runs fp8 DoubleRow over
            i-chunk PAIRS: e tiles are [128, 2, 512] fp8 (middle dim = the two
            i-chunks of the pair), V is fp8 at x64, so each attnV matmul
            contracts 256 i's in one 512-column pass. The exp stream is split:
            most tiles on the Scalar engine (Exp -> fp8 out), a slice on the
            DVE via the e4m3 bit-trick int8(x*EXP_A + EXP_B) whose saturation
            at -128 doubles as an underflow clamp (-0.0). hooks_by_sweep
            [(pr, sweep)][ic] is a list of thunks; pending pops run BEFORE
            each iteration's scores to fill the exp wait."""
            from collections import deque

            pending = deque()   # (eT_pair, m, pr, acc, sweep), newest at right
            norm_q = deque()    # (ready_gic, pr, ctx, sweep): deferred stage2
            gic = [0]           # global ic counter across all blocks
            exp_n = [0]         # global exp-tile counter for ACT/DVE split

            def flush_norms(limit_gic=None):
                while norm_q and (limit_gic is None or norm_q[0][0] <= limit_gic):
                    _, npr, nctx, nsweep = norm_q.popleft()
                    normalize_stage2(npr, nctx, nsweep)

            def do_attnv(peT, m, ppr, pacc, psweep):
                for hh in range(2):
                    mm(
                        pacc[hh][:],
                        v_sb[:, 2 * m : 2 * m + 2, 2 * ppr + hh, :],
                        peT[hh][:],
                        start=(m == 0),
                        stop=(m == NIC // 2 - 1),
                        perf_mode=DR,
                    )

            for pr in range(NPAIR):
                for sweep in range(2):
                    j0 = sweep * 512
                    acc = [
                        psAcc.tile(
                            [DV + 1, 512], F32, tag="acc", name=f"acc{sweep}_{pr}_{hh}"
                        )
                        for hh in range(2)
                    ]
                    hooks = hooks_by_sweep.get((pr, sweep), {})
                    # Deep attnV lag (in ic-pair units): an attnV pop trails
                    # its exps by ~6 pairs, so neither ACT/DVE exp latency nor
                    # queueing ever stalls the PE. psAcc's 2 buffers stay
                    # consistent: block B's pops drain during block B+1, whose
                    # own pops drain during B+2, steady-state.
                    limit = 7 if pr == 0 else 6
                    cur = None
                    for ic in range(NIC):
                        gic[0] += 1
                        flush_norms(gic[0])
                        for fn in hooks.get(ic, ()):
                            fn()
                        while len(pending) >= limit:
                            pa = pending.popleft()
                            do_attnv(*pa)
                            if pa[1] == NIC // 2 - 1:
                                ctx = normalize_stage1(pa[2], pa[3], pa[4])
                                norm_q.append((gic[0] + 3, pa[2], ctx, pa[4]))
                        # scores^T, row-tiled pair (K=64 at partitions 0/64);
                        # one 1-bank PSUM slot per (ic, head); the exp writes
                        # the matching plane of the DR-ready [128,2,512] tile.
                        # hh0 exps on ACT, hh1 on DVE for 3 of 4 pairs.
                        half = ic % 2
                        if half == 0:
                            cur = [
                                pe_pool.tile(
                                    [128, 2, 512], F8, tag="eT",
                                    name=f"eT{sweep}_{pr}_{ic // 2}_{hh}",
                                )
                                for hh in range(2)
                            ]
                            exp_n[0] += 1
                        dve_hh1 = exp_n[0] % DVE_PAIR_MOD < DVE_PAIR_CNT
                        for hh in range(2):
                            ps_s = psS.tile(
                                [128, 512], F32, tag="sc",
                                name=f"ps_s{sweep}_{pr}_{ic}_{hh}",
                            )
                            mm(
                                ps_s[:],
                                qT_sb[hh * 64 : (hh + 1) * 64, pr, ic * 128 : (ic + 1) * 128],
                                kT_sb[hh * 64 : (hh + 1) * 64, pr, j0 : j0 + 512],
                                start=True,
                                stop=True,
                            )
                            dst = cur[hh][:, half, :]
                            if hh == 1 and dve_hh1:
                                nc.vector.tensor_scalar(
                                    out=dst.bitcast(I8),
                                    in0=ps_s[:],
                                    scalar1=EXP_A,
                                    scalar2=EXP_B,
                                    op0=mult,
                                    op1=add,
                                )
                            else:
                                # x256-scaled q, k: 2^-16 inside EXP_SCALE
                                nc.scalar.activation(
                                    dst, ps_s[:], Exp, scale=EXP_SCALE
                                )
                        if half == 1:
                            pending.append((cur, ic // 2, pr, acc, sweep))
                    if after_pair and (sweep, pr) in after_pair:
                        after_pair[(sweep, pr)]()
            while pending:
                pa = pending.popleft()
                do_attnv(*pa)
                if pa[1] == NIC // 2 - 1:
                    ctx = normalize_stage1(pa[2], pa[3], pa[4])
                    norm_q.append((0, pa[2], ctx, pa[4]))
            flush_norms()

        # Hook schedule for pair-major order (pair p: j-half-0 block then
        # j-half-1 block). Each pair's j-half-1 block has no inherent hook
        # needs, so it carries the NEXT pair's q t0 / k chains and a share of
        # the deferred V projections — load stays near-uniform across all 256
        # iterations instead of crowding a single global sweep.
        hooks = {}

        def add_hook(pr, sweep, ic, fn):
            hooks.setdefault((pr, sweep), {}).setdefault(ic, []).append(fn)

        # V heads 0-7 (pairs 0-3): i-chunks 0-9 in pair 0's first block, the
        # lag-6 tail (i-chunks 10-15) in its second block just before each is
        # consumed. V heads 8-15 (first used at pair 4) spread over pairs
        # 1-3's second blocks.
        # one hook per iteration (chains own ics 3/5/7/11); every V keeps a
        # >=2-iteration lead on its lag-7 attnV pop (v_m pops at ic m+7)
        add_hook(0, 0, 1, lambda: proj_v(0, 0))
        add_hook(0, 0, 1, lambda: proj_v(1, 0))
        for ic, m in ((2, 2), (4, 3), (9, 4), (10, 5), (6, 6), (12, 7), (8, 8)):
            add_hook(0, 0, ic, lambda m=m: proj_v(m, 0))
        add_hook(0, 0, 14, lambda: proj_v(9, 0))
        add_hook(0, 0, 15, lambda: proj_v(10, 0))
        for k in range(5):
            add_hook(0, 1, k, lambda k=k: proj_v(11 + k, 0))
        for k, ic in enumerate((1, 3, 5, 13, 14, 15)):
            add_hook(1, 1, ic, lambda k=k: proj_v(k, 1))
        for k, ic in enumerate((1, 2, 3, 13, 14)):
            add_hook(2, 1, ic, lambda k=k: proj_v(6 + k, 1))
        for k, ic in enumerate((1, 2, 3)):
            add_hook(3, 1, ic, lambda k=k: proj_v(11 + k, 1))
        add_hook(4, 0, 12, lambda: proj_v(14, 1))
        add_hook(4, 0, 13, lambda: proj_v(15, 1))
        for pr in range(NPAIR):
            # own q t1-3, consumed by this pair's first block from ic4/8/12 on
            for t, ic in ((1, 3), (2, 7), (3, 11)):
                add_hook(pr, 0, ic, lambda pr=pr, t=t: proj_chain(pr, "q", t))
        add_hook(0, 0, 5, lambda: proj_chain(0, "k", 1))
        for pr in range(7):  # next pair's bootstrap rides the j-half-1 block
            add_hook(pr, 1, 7, lambda pr=pr: proj_chain(pr + 1, "q", 0))
            add_hook(pr, 1, 9, lambda pr=pr: proj_chain(pr + 1, "k", 0))
            add_hook(pr, 1, 11, lambda pr=pr: proj_chain(pr + 1, "k", 1))

        pc_tiles = {}

        def open_phase_c():
            # pa's tensors are all dead once sweep 0 ends; reuse the space for
            # phase C inputs so their DMA overlaps all of sweep 1.
            pa.release()
            pc = stack.enter_context(tc.tile_pool(name="pc", bufs=1))
            pc_tiles["wo"] = pc.tile([128, 2, NKC, 512], F8, name="wo_sb")
            pc_tiles["tokres"] = pc.tile([128, NJCH, D], F32, name="tokres_sb")
            nc.sync.dma_start(pc_tiles["wo"][:], wo_d[:])
            nc.sync.dma_start(pc_tiles["tokres"][:], tokres_d[:])

        # upfront: just enough projection for sweep-0 pair-0's first scores
        proj_chain(0, "q", 0)
        proj_chain(0, "k", 0)

        attention(hooks, after_pair={(0, 7): open_phase_c})
        wo_sb = pc_tiles["wo"]
        tokres_sb = pc_tiles["tokres"]
        paJ.release()
        psAcc.release()
        psS.release()
        # ---------------- Phase C: out-proj + residual + LayerNorm ----------------
        with (
            tc.tile_pool(name="pC", bufs=4) as pC,
            tc.tile_pool(name="pStats", bufs=8) as pStats,
            tc.tile_pool(name="psC", bufs=4, space="PSUM") as psC,
        ):
            # Out-proj in two steps per jch: kc 0-6 accumulate early (their
            # multiT chunks are ready pairs before the last normalize), kc 7
            # finishes when multiT[7] lands. Prefilling 4 PSUM groups hides
            # the last normalize's DRAM round-trip behind ~12us of matmuls.
            prefill = {}

            def emit_prefill(jch):
                ps_o = psC.tile([128, D], F32, tag="po", name=f"ps_o{jch}")
                for kcp in range(NKC // 2 - 1):
                    lhsT = multiT[kcp][:, :, jch * 128 : (jch + 1) * 128]
                    for nb in range(2):
                        mm(
                            ps_o[:, nb * 512 : (nb + 1) * 512],
                            lhsT,
                            wo_sb[:, nb, 2 * kcp : 2 * kcp + 2, :],
                            start=(kcp == 0),
                            stop=False,
                            perf_mode=DR,
                        )
                prefill[jch] = ps_o

            for jch in range(4):
                emit_prefill(jch)
            for jch in range(NJCH):
                ps_o = prefill.pop(jch)
                lhsT = multiT[NKC // 2 - 1][:, :, jch * 128 : (jch + 1) * 128]
                for nb in range(2):
                    mm(
                        ps_o[:, nb * 512 : (nb + 1) * 512],
                        lhsT,
                        wo_sb[:, nb, NKC - 2 : NKC, :],
                        start=False,
                        stop=True,
                        perf_mode=DR,
                    )
                # x = psum + residual, sum_t = rowsum(x), in one DVE pass
                x_sb = pC.tile([128, D], F32, tag="x", name=f"x{jch}")
                sum_t = pStats.tile([128, 1], F32, tag="sum", name=f"sum{jch}")
                # x = psum/WSCALE + residual (undo the Wo fp8 pre-scale),
                # sum_t = rowsum(x), in one DVE pass
                nc.vector.scalar_tensor_tensor(
                    out=x_sb[:],
                    in0=ps_o[:],
                    scalar=1.0 / WSCALE,
                    in1=tokres_sb[:, jch, :],
                    op0=mult,
                    op1=add,
                    accum_out=sum_t[:],
                )
                negmean = pStats.tile([128, 1], F32, tag="nm", name=f"nm{jch}")
                nc.vector.tensor_scalar_mul(negmean[:], sum_t[:], -1.0 / D)
                # Variance always on Scalar (Square w/ accum); the final scale
                # alternates ACT/DVE by jch parity -- balances the two queues
                # at the drain with fewer cross-engine hops per chain.
                scrap = pC.tile([128, D], BF, tag="scrap", name=f"scrap{jch}")
                ssq = pStats.tile([128, 1], F32, tag="ssq", name=f"ssq{jch}")
                nc.scalar.activation(
                    scrap[:], x_sb[:], Square, bias=negmean[:], accum_out=ssq[:]
                )
                std_t = pStats.tile([128, 1], F32, tag="std", name=f"std{jch}")
                nc.scalar.activation(std_t[:], ssq[:], Sqrt, bias=eps_sb[:], scale=1.0 / D)
                rstd = pStats.tile([128, 1], F32, tag="rstd", name=f"rstd{jch}")
                nc.vector.reciprocal(rstd[:], std_t[:])
                # (x - m) * rstd == x*rstd + (negmean*rstd), one fused op
                rstd_nm = pStats.tile([128, 1], F32, tag="rnm", name=f"rnm{jch}")
                nc.vector.tensor_tensor(rstd_nm[:], negmean[:], rstd[:], mult)
                out_sb = pC.tile([128, D], F32, tag="out", name=f"out{jch}")
                if jch % 2 == 0:
                    nc.scalar.activation(
                        out_sb[:],
                        x_sb[:],
                        mybir.ActivationFunctionType.Identity,
                        bias=rstd_nm[:],
                        scale=rstd[:],
                    )
                else:
                    nc.vector.tensor_scalar(
                        out=out_sb[:], in0=x_sb[:], scalar1=rstd[:],
                        scalar2=rstd_nm[:], op0=mult, op1=add,
                    )
                if apply_affine:
                    nc.gpsimd.tensor_tensor(out_sb[:], out_sb[:], gamma_sb[:], mult)
                    nc.gpsimd.tensor_tensor(out_sb[:], out_sb[:], beta_sb[:], add)
                nc.sync.dma_start(out_d[:, jch], out_sb[:])
                # second prefill wave once the first four STTs are emitted, so
                # the PE chain never parks on a not-yet-freed PSUM group
                if jch == 3:
                    for j2 in range(4, NJCH):
                        emit_prefill(j2)

    nc.compile()
    return nc


def _prep_inputs(tokens, Wq, Wk, Wv, Wo, gamma, beta):
    """Host-side layout prep. Returns per-core input maps. All tensors use
    block-major layouts so every kernel DMA is contiguous per partition:
    [p, blk, kc, cols] with contraction row index kc*128+p."""
    tokens = np.ascontiguousarray(np.asarray(tokens, dtype=np.float32))

    def blocks(a, ncols):  # [1024, N] -> [128, N//ncols, NKC, ncols]
        return np.ascontiguousarray(
            a.reshape(NKC, 128, a.shape[-1] // ncols, ncols).transpose(1, 2, 0, 3)
        )

    wq_all = blocks(
        (np.asarray(Wq).transpose(1, 0, 2).reshape(D, H * DK) * WSCALE).astype(FP8),
        128,
    )
    wk_all = blocks(
        (np.asarray(Wk).transpose(1, 0, 2).reshape(D, H * DK) * WSCALE).astype(FP8),
        128,
    )
    wv_all = blocks(
        (np.asarray(Wv).transpose(1, 0, 2).reshape(D, H * DV) * WSCALE).astype(FP8),
        512,
    )
    wo_all = blocks((np.asarray(Wo) * WSCALE).astype(FP8), 512)
    gamma_bc = np.ascontiguousarray(
        np.broadcast_to(np.asarray(gamma, np.float32), (128, D))
    )
    beta_bc = np.ascontiguousarray(
        np.broadcast_to(np.asarray(beta, np.float32), (128, D))
    )

    tokT_by_b = []
    for b in range(B):
        tokT_by_b.append(blocks(tokens[b].T.astype(FP8), 512))  # [128,4,8,512]

    in_maps = []
    for c in range(NCORES):
        b, jc = c // 2, c % 2
        tokT = tokT_by_b[b]
        tokTj = np.ascontiguousarray(tokT[:, 2 * jc : 2 * jc + 2])
        tokres = np.ascontiguousarray(
            tokens[b, jc * JW : (jc + 1) * JW]
            .reshape(NJCH, 128, D)
            .transpose(1, 0, 2)
        )
        in_maps.append(
            {
                "tokT": tokT,
                "tokTj": tokTj,
                "wq": wq_all,
                "wk": wk_all,
                "wv": wv_all,
                "wo": wo_all,
                "tokres": tokres,
                "gamma_bc": gamma_bc,
                "beta_bc": beta_bc,
            }
        )
    return in_maps


def run(inputs, trace=False, tmpdir=None):
    """Run on hardware; returns (output, BassKernelResults)."""
    from concourse.bass_utils import run_bass_kernel_spmd

    apply_affine = not (
        np.all(np.asarray(inputs["gamma"]) == 1.0)
        and np.all(np.asarray(inputs["beta"]) == 0.0)
    )
    key = ("nc", apply_affine)
    if key not in _CACHE:
        _CACHE[key] = _build_nc(apply_affine)
    nc = _CACHE[key]
    in_maps = _prep_inputs(**inputs)
    res = run_bass_kernel_spmd(
        nc, in_maps, core_ids=list(range(NCORES)), trace=trace, tmpdir=tmpdir
    )
    out = np.empty((B, S, D), np.float32)
    for c in range(NCORES):
        b, jc = c // 2, c % 2
        o = res.results[c]["out"]  # [128, 8, 1024]
        out[b, jc * JW : (jc + 1) * JW] = (
            o.transpose(1, 0, 2).reshape(JW, D)
        )
    return out, res


def kernel(tokens, Wq, Wk, Wv, Wo, gamma, beta):
    out, _ = run(
        dict(tokens=tokens, Wq=Wq, Wk=Wk, Wv=Wv, Wo=Wo, gamma=gamma, beta=beta)
    )
    return out



# revision 92
# speedup vs baseline: 1.3234x; 1.0123x over previous
"""TRN2 Bass kernel for fused MHA (softmax-over-query quirk) + out-proj + residual + LayerNorm.

Problem shapes (hardcoded): tokens [4,2048,1024], Wq/Wk [16,1024,64], Wv [16,1024,64],
Wo [1024,1024], gamma/beta [1024]. Output [4,2048,1024] fp32.

Sharding: 8 cores, core c owns (batch b=c//2, S-half jc=c%2) of the OUTPUT rows.
No collectives. Each core computes, for its batch b:
  qT[dk,i] (full S), kT[dk,j] (its half) in bf16, V[i,dv] in fp8 (x64),
  scores^T[i,j] = q_i.k_j (PSUM fp32), e = exp(scores/8) in fp8e4m3,
  heads^T[dv,j] + rowsum row via a x64 ones-column appended to V,
  multi^T = heads^T / rowsum, out = multi @ Wo + tokens, LayerNorm rows.

All four projections AND the attention-value contraction run in fp8e4m3
DoubleRow (2 K-planes per matmul): tokens/multi cast to fp8, weights x256
(dodges e4m3 subnormals; the scales cancel exactly through softmax and the
out-proj's 2^8 is divided out in the residual-add STT). attnV pairs two
i-chunks per pass: each exp writes one plane of a DoubleRow-ready
[128, 2, 512] fp8 e-tile, V is stored [i, ic, h, dv|ones] so v_sb[:,2m:2m+2,h,:]
is the matching [128,2,65] stationary block -- attnV matmul count halves.

The exp stream (the former bottleneck) is split across engines: hh0 exps on
Scalar (Exp -> fp8 out, same cost as bf16), hh1 exps on the DVE for 3 of 4
ic-pairs via the e4m3 bit-trick int8(round(x*8*log2e + 56)) bitcast to fp8
-- one tensor_scalar op whose int8 write-saturation doubles as an underflow
clamp (bits -128 = -0.0). The mod-4 pattern keeps each block's last pair
all-Scalar so block-boundary deps never queue behind the DVE.

Schedule: pair-major blocks (pr, sweep) x 16 i-chunks; projections ride as
hooks in iteration slots (each pair's j-half-1 block carries the next
pair's q/k chains and V projections). attnV pops trail their exps by 6-7
ic-PAIRS (deep lag: exp latency/queueing never stalls the PE; psAcc's two
banks stay consistent with pops draining one block late). Normalize is
split: pop-time stage1 (acc->SBUF drain + rowsum DRAM-broadcast roundtrip),
stage2 (reciprocal + GpSimd mults) deferred 3 iterations so the in-order
DVE queue never parks on the roundtrip. Matmuls are NOT chained (the
scheduler fills stalls). DRAM layouts are block-major so every DMA is
contiguous per partition. Phase C: fp8-DR out-proj with 4+4 PSUM prefill
waves, residual+LN with variance on Scalar and the final scale alternating
Scalar/DVE. Measured ~325us on 8 cores (prior session 380us, stub 513us),
rel err 8.2e-4 (gate 2e-2).
"""

import numpy as np
import ml_dtypes

BF16 = ml_dtypes.bfloat16
FP8 = ml_dtypes.float8_e4m3

B, S, D, H, DK, DV = 4, 2048, 1024, 16, 64, 64
NCORES = 8
NPAIR = 8     # head pairs
NKC = 8       # D // 128 contraction chunks
NIC = 16      # S // 128 i-chunks
JW = 1024     # j columns per core (S/2)
NJCH = 8      # JW // 128
LN_EPS = 1e-5
WSCALE = 256.0  # fp8 weight pre-scale (power of 2)
VSCALE = 64.0   # on-chip V scale: psum(x256) * 0.25 -> fp8 with no clipping
EXP_SCALE = 0.125 / (WSCALE * WSCALE)  # true scores/8 per PSUM unit
LOG2E = 1.4426950408889634
# DVE exp bit-trick: int8(round(x*EXP_A + EXP_B)) viewed as e4m3 ~= exp(x*EXP_SCALE)
EXP_A = 8.0 * LOG2E * EXP_SCALE
EXP_B = 56.0  # e4m3 exponent bias 7 << 3 mantissa bits
DVE_PAIR_MOD = 4   # of every 4 ic-pairs, this many run their hh1 exp on the DVE
DVE_PAIR_CNT = 3   # (hh0 always on ACT: split pairs have latency max, not sum)

_CACHE = {}


def _build_nc(apply_affine):
    import concourse.tile as tile
    from concourse import bacc, mybir

    F32 = mybir.dt.float32
    BF = mybir.dt.bfloat16
    F8 = mybir.dt.float8e4
    I8 = mybir.dt.int8
    Exp = mybir.ActivationFunctionType.Exp
    Copy = mybir.ActivationFunctionType.Copy
    Square = mybir.ActivationFunctionType.Square
    Sqrt = mybir.ActivationFunctionType.Sqrt
    mult = mybir.AluOpType.mult
    add = mybir.AluOpType.add
    DR = mybir.MatmulPerfMode.DoubleRow

    nc = bacc.Bacc(
        "TRN2",
        target_bir_lowering=False,
        debug=False,
        enable_asserts=False,
        num_devices=NCORES,
    )

    # DRAM I/O (per-core views; host prepares layouts). Block-major orders
    # (i-blocks of 512 for tokens, head-pair blocks for Wq/Wk, head-half
    # blocks for Wv/Wo) make every DMA the kernel issues contiguous per
    # partition -- high descriptor efficiency, fast startup ramp.
    tokT_d = nc.dram_tensor("tokT", (128, 4, NKC, 512), F8, kind="ExternalInput").ap()
    tokTj_d = nc.dram_tensor("tokTj", (128, 2, NKC, 512), F8, kind="ExternalInput").ap()
    wq_d = nc.dram_tensor("wq", (128, NPAIR, NKC, 128), F8, kind="ExternalInput").ap()
    wk_d = nc.dram_tensor("wk", (128, NPAIR, NKC, 128), F8, kind="ExternalInput").ap()
    wv_d = nc.dram_tensor("wv", (128, 2, NKC, 512), F8, kind="ExternalInput").ap()
    wo_d = nc.dram_tensor("wo", (128, 2, NKC, 512), F8, kind="ExternalInput").ap()
    tokres_d = nc.dram_tensor("tokres", (128, NJCH, D), F32, kind="ExternalInput").ap()
    if apply_affine:
        gamma_d = nc.dram_tensor("gamma_bc", (128, D), F32, kind="ExternalInput").ap()
        beta_d = nc.dram_tensor("beta_bc", (128, D), F32, kind="ExternalInput").ap()
    out_d = nc.dram_tensor("out", (128, NJCH, D), F32, kind="ExternalOutput").ap()
    from contextlib import ExitStack

    from concourse.bass import _add_dep_helper

    # Chain all PE matmuls in emission order: stops the scheduler from
    # interleaving row-conflicting matmuls and keeps the stream dense.
    CHAIN_MM = False
    _prev_mm = [None]

    def mm(*args, **kwargs):
        inst = nc.tensor.matmul(*args, **kwargs)
        if CHAIN_MM and _prev_mm[0] is not None:
            _add_dep_helper(inst.ins, _prev_mm[0].ins, sync=False, reason="pe-order")
        _prev_mm[0] = inst
        return inst

    with tile.TileContext(nc) as tc, ExitStack() as stack:
        persist = stack.enter_context(tc.tile_pool(name="persist", bufs=1))
        qT_sb = persist.tile([128, NPAIR, S], BF)          # [pair-dk, pr, i]
        kT_sb = persist.tile([128, NPAIR, JW], BF)         # [pair-dk, pr, j]
        v_sb = persist.tile([128, NIC, H, DV + 1], F8)     # [i%128, ic, h, dv|64s]
        # multi^T in fp8, one tile per KC-PAIR ([128, 2, JW], middle dim = the
        # DoubleRow K-plane) so the out-proj runs fp8 DoubleRow; per-pair-ish
        # tiles keep out-proj dep tracking from serializing on the last write
        multiT = [
            persist.tile([128, 2, JW], F8, name=f"multiT{i}") for i in range(NKC // 2)
        ]
        eps_sb = persist.tile([128, 1], F32)
        if apply_affine:
            gamma_sb = persist.tile([128, D], F32)
            beta_sb = persist.tile([128, D], F32)
            nc.sync.dma_start(gamma_sb[:], gamma_d[:])
            nc.sync.dma_start(beta_sb[:], beta_d[:])
        nc.vector.memset(eps_sb[:], LN_EPS)
        for ic in range(NIC):
            # ones column scaled by VSCALE so rowsum matches the x64 V scale
            nc.vector.memset(v_sb[:, ic, :, DV : DV + 1], VSCALE)

        # pools that outlive pa must be allocated first (LIFO release)
        # psS: six 1-bank [128,512] slots shared by the scores ring and the
        # projection-chain borrows -- deep enough that neither a scores matmul
        # nor a proj chain ever waits on a slot whose exp hasn't fired yet.
        psS = tc.alloc_tile_pool(name="psS", bufs=6, space="PSUM")
        psAcc = tc.alloc_tile_pool(name="psAcc", bufs=2, space="PSUM")
        pe_pool = stack.enter_context(tc.tile_pool(name="pe", bufs=22))
        pn_pool = stack.enter_context(tc.tile_pool(name="pn", bufs=6))
        pdram = stack.enter_context(tc.tile_pool(name="pdram", bufs=6, space="DRAM"))

        # tokTj and wk outlive pa (the k j-half-1 chains consume them in
        # sweep 1), so they live in their own right-side pool.
        paJ = tc.alloc_tile_pool(name="paJ", bufs=1, side="right")
        tokTj_sb = paJ.tile([128, 2, NKC, 512], F8)
        wk_sb = paJ.tile([128, NPAIR, NKC, 128], F8)
        pa = tc.alloc_tile_pool(name="pa", bufs=1)
        wq_sb = pa.tile([128, NPAIR, NKC, 128], F8)
        tokT_sb = pa.tile([128, 4, NKC, 512], F8)
        wv_sb = pa.tile([128, 2, NKC, 512], F8)

        # Startup DMA over both HWDGE rings (SP + ACT, independent FIFOs).
        # Every transfer below is contiguous per partition. Critical path to
        # the first exp: wq pair-0 + tokens i-block-0 + wk pair-0 + tokTj
        # j-half-0; everything else queues behind.
        nc.sync.dma_start(wq_sb[:, 0], wq_d[:, 0])
        nc.scalar.dma_start(tokT_sb[:, 0], tokT_d[:, 0])
        nc.sync.dma_start(wk_sb[:, 0], wk_d[:, 0])
        nc.sync.dma_start(tokTj_sb[:, 0], tokTj_d[:, 0])
        nc.scalar.dma_start(wv_sb[:], wv_d[:])
        nc.sync.dma_start(tokTj_sb[:, 1], tokTj_d[:, 1])
        nc.scalar.dma_start(tokT_sb[:, 1], tokT_d[:, 1])
        nc.sync.dma_start(tokT_sb[:, 2], tokT_d[:, 2])
        nc.sync.dma_start(tokT_sb[:, 3], tokT_d[:, 3])
        nc.sync.dma_start(wq_sb[:, 1:], wq_d[:, 1:])
        nc.sync.dma_start(wk_sb[:, 1:], wk_d[:, 1:])

        def proj_chain(pr, which, t):
            """One 512-wide fp8 DoubleRow projection chain via a borrowed
            scores-pool slot."""
            w_sb, dst, rhs_sb = (
                (wq_sb, qT_sb, tokT_sb) if which == "q" else (wk_sb, kT_sb, tokTj_sb)
            )
            ps = psS.tile([128, 512], F32, tag="sc", name=f"pj{which}{pr}_{t}")
            for kc in range(0, NKC, 2):
                mm(
                    ps[:],
                    w_sb[:, pr, kc : kc + 2, :],
                    rhs_sb[:, t, kc : kc + 2, :],
                    start=(kc == 0),
                    stop=(kc == NKC - 2),
                    perf_mode=DR,
                )
            nc.vector.tensor_copy(out=dst[:, pr, t * 512 : (t + 1) * 512], in_=ps[:])

        def proj_v(ic, nb):
            """fp8 DoubleRow V projection for one i-chunk and one 8-head half
            via a borrowed scores-pool slot. PSUM is x256; scale by 0.25 into
            fp8 (std ~41, max ~5 sigma = 205 << 448: no clipping)."""
            for nb in (nb,):
                ps = psS.tile([128, 512], F32, tag="sc", name=f"pjv{ic}_{nb}")
                for kc in range(0, NKC, 2):
                    mm(
                        ps[:],
                        tokT_sb[:, ic // 4, kc : kc + 2,
                                (ic % 4) * 128 : (ic % 4 + 1) * 128],
                        wv_sb[:, nb, kc : kc + 2, :],
                        start=(kc == 0),
                        stop=(kc == NKC - 2),
                        perf_mode=DR,
                    )
                nc.vector.tensor_scalar_mul(
                    v_sb[:, ic, nb * 8 : (nb + 1) * 8, 0:DV],
                    ps.rearrange("p (h v) -> p h v", h=8),
                    VSCALE / WSCALE,
                )

        def normalize_stage1(pr, acc, sweep):
            """Pop-time half of multi^T = heads^T / rowsum: drain acc to SBUF
            (both heads' copies back-to-back, keeping the DVE queue clear of
            roundtrip-blocked ops) and launch the rowsum DRAM broadcast."""
            ctx = []
            for hh in range(2):
                h = 2 * pr + hh
                hraw = pn_pool.tile(
                    [DV + 1, 512], F32, tag="hraw", name=f"hraw{sweep}_{h}"
                )
                nc.vector.tensor_copy(out=hraw[:], in_=acc[hh][:])  # frees acc
                ctx.append(hraw)
            for hh in range(2):
                h = 2 * pr + hh
                rs_dram = pdram.tile([1, 512], F32, tag="rsd", name=f"rsd{sweep}_{h}")
                nc.sync.dma_start(out=rs_dram[:], in_=ctx[hh][DV : DV + 1, :])
                rec_in = pn_pool.tile([DV, 512], F32, tag="rin", name=f"rin{sweep}_{h}")
                nc.gpsimd.dma_start(out=rec_in[:], in_=rs_dram.to_broadcast((DV, 512)))
                ctx.append(rec_in)
            return ctx

        def normalize_stage2(pr, ctx, sweep):
            """Deferred half: recips run with their broadcast long landed, so
            they never block the in-order DVE queue."""
            j0 = sweep * 512
            for hh in range(2):
                hraw, rec_in = ctx[hh], ctx[2 + hh]
                nc.vector.reciprocal_approx_fast(out=rec_in[:], in_=rec_in[:])
                if hh == 0:
                    nc.gpsimd.tensor_tensor(
                        multiT[pr // 2][0:64, pr % 2, j0 : j0 + 512],
                        hraw[0:DV, :], rec_in[:], mult,
                    )
                else:
                    tmp64 = pn_pool.tile(
                        [DV, 512], F8, tag="tmp64", name=f"tmp{sweep}_{2 * pr + hh}"
                    )
                    nc.gpsimd.tensor_tensor(tmp64[:], hraw[0:DV, :], rec_in[:], mult)
                    nc.sync.dma_start(
                        out=multiT[pr // 2][64:128, pr % 2, j0 : j0 + 512],
                        in_=tmp64[:],
                    )

        def attention(hooks_by_sweep, after_pair=None):
            """Two j-half sweeps over all pairs. attnV runs fp8 DoubleRow over
            i-chunk PAIRS: e tiles are [128, 2, 512] fp8 (middle dim = the two
            i-chunks of the pair), V is fp8 at x64, so each attnV matmul
            contracts 256 i's in one 512-column pass. The exp stream is split:
            most tiles on the Scalar engine (Exp -> fp8 out), a slice on the
            DVE via the e4m3 bit-trick int8(x*EXP_A + EXP_B) whose saturation
            at -128 doubles as an underflow clamp (-0.0). hooks_by_sweep
            [(pr, sweep)][ic] is a list of thunks; pending pops run BEFORE
            each iteration's scores to fill the exp wait."""
            from collections import deque

            pending = deque()   # (eT_pair, m, pr, acc, sweep), newest at right
            norm_q = deque()    # (ready_gic, pr, ctx, sweep): deferred stage2
            gic = [0]           # global ic counter across all blocks
            exp_n = [0]         # global exp-tile counter for ACT/DVE split

            def flush_norms(limit_gic=None):
                while norm_q and (limit_gic is None or norm_q[0][0] <= limit_gic):
                    _, npr, nctx, nsweep = norm_q.popleft()
                    normalize_stage2(npr, nctx, nsweep)

            def do_attnv(peT, m, ppr, pacc, psweep):
                for hh in range(2):
                    mm(
                        pacc[hh][:],
                        v_sb[:, 2 * m : 2 * m + 2, 2 * ppr + hh, :],
                        peT[hh][:],
                        start=(m == 0),
                        stop=(m == NIC // 2 - 1),
                        perf_mode=DR,
                    )

            for pr in range(NPAIR):
                for sweep in range(2):
                    j0 = sweep * 512
                    acc = [
                        psAcc.tile(
                            [DV + 1, 512], F32, tag="acc", name=f"acc{sweep}_{pr}_{hh}"
                        )
                        for hh in range(2)
                    ]
                    hooks = hooks_by_sweep.get((pr, sweep), {})
                    # Deep attnV lag (in ic-pair units): an attnV pop trails
                    # its exps by ~6 pairs, so neither ACT/DVE exp latency nor
                    # queueing ever stalls the PE. psAcc's 2 buffers stay
                    # consistent: block B's pops drain during block B+1, whose
                    # own pops drain during B+2, steady-state.
                    limit = 7 if pr == 0 else 6
                    cur = None
                    for ic in range(NIC):
                        gic[0] += 1
                        flush_norms(gic[0])
                        for fn in hooks.get(ic, ()):
                            fn()
                        while len(pending) >= limit:
                            pa = pending.popleft()
                            do_attnv(*pa)
                            if pa[1] == NIC // 2 - 1:
                                ctx = normalize_stage1(pa[2], pa[3], pa[4])
                                norm_q.append((gic[0] + 3, pa[2], ctx, pa[4]))
                        # scores^T, row-tiled pair (K=64 at partitions 0/64);
                        # one 1-bank PSUM slot per (ic, head); the exp writes
                        # the matching plane of the DR-ready [128,2,512] tile.
                        # hh0 exps on ACT, hh1 on DVE for 3 of 4 pairs.
                        half = ic % 2
                        if half == 0:
                            cur = [
                                pe_pool.tile(
                                    [128, 2, 512], F8, tag="eT",
                                    name=f"eT{sweep}_{pr}_{ic // 2}_{hh}",
                                )
                                for hh in range(2)
                            ]
                            exp_n[0] += 1
                        dve_hh1 = exp_n[0] % DVE_PAIR_MOD < DVE_PAIR_CNT
                        for hh in range(2):
                            ps_s = psS.tile(
                                [128, 512], F32, tag="sc",
                                name=f"ps_s{sweep}_{pr}_{ic}_{hh}",
                            )
                            mm(
                                ps_s[:],
                                qT_sb[hh * 64 : (hh + 1) * 64, pr, ic * 128 : (ic + 1) * 128],
                                kT_sb[hh * 64 : (hh + 1) * 64, pr, j0 : j0 + 512],
                                start=True,
                                stop=True,
                            )
                            dst = cur[hh][:, half, :]
                            if hh == 1 and dve_hh1:
                                nc.vector.tensor_scalar(
                                    out=dst.bitcast(I8),
                                    in0=ps_s[:],
                                    scalar1=EXP_A,
                                    scalar2=EXP_B,
                                    op0=mult,
                                    op1=add,
                                )
                            else:
                                # x256-scaled q, k: 2^-16 inside EXP_SCALE
                                nc.scalar.activation(
                                    dst, ps_s[:], Exp, scale=EXP_SCALE
                                )
                        if half == 1:
                            pending.append((cur, ic // 2, pr, acc, sweep))
                    if after_pair and (sweep, pr) in after_pair:
                        after_pair[(sweep, pr)]()
            while pending:
                pa = pending.popleft()
                do_attnv(*pa)
                if pa[1] == NIC // 2 - 1:
                    ctx = normalize_stage1(pa[2], pa[3], pa[4])
                    norm_q.append((0, pa[2], ctx, pa[4]))
            flush_norms()

        # Hook schedule for pair-major order (pair p: j-half-0 block then
        # j-half-1 block). Each pair's j-half-1 block has no inherent hook
        # needs, so it carries the NEXT pair's q t0 / k chains and a share of
        # the deferred V projections — load stays near-uniform across all 256
        # iterations instead of crowding a single global sweep.
        hooks = {}

        def add_hook(pr, sweep, ic, fn):
            hooks.setdefault((pr, sweep), {}).setdefault(ic, []).append(fn)

        # V heads 0-7 (pairs 0-3): i-chunks 0-9 in pair 0's first block, the
        # lag-6 tail (i-chunks 10-15) in its second block just before each is
        # consumed. V heads 8-15 (first used at pair 4) spread over pairs
        # 1-3's second blocks.
        # one hook per iteration (chains own ics 3/5/7/11); every V keeps a
        # >=2-iteration lead on its lag-7 attnV pop (v_m pops at ic m+7)
        add_hook(0, 0, 1, lambda: proj_v(0, 0))
        add_hook(0, 0, 1, lambda: proj_v(1, 0))
        for ic, m in ((2, 2), (4, 3), (9, 4), (10, 5), (6, 6), (12, 7), (8, 8)):
            add_hook(0, 0, ic, lambda m=m: proj_v(m, 0))
        add_hook(0, 0, 14, lambda: proj_v(9, 0))
        add_hook(0, 0, 15, lambda: proj_v(10, 0))
        for k in range(5):
            add_hook(0, 1, k, lambda k=k: proj_v(11 + k, 0))
        for k, ic in enumerate((1, 3, 5, 13, 14, 15)):
            add_hook(1, 1, ic, lambda k=k: proj_v(k, 1))
        for k, ic in enumerate((1, 2, 3, 13, 14)):
            add_hook(2, 1, ic, lambda k=k: proj_v(6 + k, 1))
        for k, ic in enumerate((1, 2, 3)):
            add_hook(3, 1, ic, lambda k=k: proj_v(11 + k, 1))
        add_hook(4, 0, 12, lambda: proj_v(14, 1))
        add_hook(4, 0, 13, lambda: proj_v(15, 1))
        for pr in range(NPAIR):
            # own q t1-3, consumed by this pair's first block from ic4/8/12 on
            for t, ic in ((1, 3), (2, 7), (3, 11)):
                add_hook(pr, 0, ic, lambda pr=pr, t=t: proj_chain(pr, "q", t))
        add_hook(0, 0, 5, lambda: proj_chain(0, "k", 1))
        for pr in range(7):  # next pair's bootstrap rides the j-half-1 block
            add_hook(pr, 1, 7, lambda pr=pr: proj_chain(pr + 1, "q", 0))
            add_hook(pr, 1, 9, lambda pr=pr: proj_chain(pr + 1, "k", 0))
            add_hook(pr, 1, 11, lambda pr=pr: proj_chain(pr + 1, "k", 1))

        pc_tiles = {}

        def open_phase_c():
            # pa's tensors are all dead once sweep 0 ends; reuse the space for
            # phase C inputs so their DMA overlaps all of sweep 1.
            pa.release()
            pc = stack.enter_context(tc.tile_pool(name="pc", bufs=1))
            pc_tiles["wo"] = pc.tile([128, 2, NKC, 512], F8, name="wo_sb")
            pc_tiles["tokres"] = pc.tile([128, NJCH, D], F32, name="tokres_sb")
            nc.sync.dma_start(pc_tiles["wo"][:], wo_d[:])
            nc.sync.dma_start(pc_tiles["tokres"][:], tokres_d[:])

        # upfront: just enough projection for sweep-0 pair-0's first scores
        proj_chain(0, "q", 0)
        proj_chain(0, "k", 0)

        attention(hooks, after_pair={(0, 7): open_phase_c})
        wo_sb = pc_tiles["wo"]
        tokres_sb = pc_tiles["tokres"]
        paJ.release()
        psAcc.release()
        psS.release()
        # ---------------- Phase C: out-proj + residual + LayerNorm ----------------
        with (
            tc.tile_pool(name="pC", bufs=4) as pC,
            tc.tile_pool(name="pStats", bufs=8) as pStats,
            tc.tile_pool(name="psC", bufs=4, space="PSUM") as psC,
        ):
            # Out-proj in two steps per jch: kc 0-6 accumulate early (their
            # multiT chunks are ready pairs before the last normalize), kc 7
            # finishes when multiT[7] lands. Prefilling 4 PSUM groups hides
            # the last normalize's DRAM round-trip behind ~12us of matmuls.
            prefill = {}

            def emit_prefill(jch):
                ps_o = psC.tile([128, D], F32, tag="po", name=f"ps_o{jch}")
                for kcp in range(NKC // 2 - 1):
                    lhsT = multiT[kcp][:, :, jch * 128 : (jch + 1) * 128]
                    for nb in range(2):
                        mm(
                            ps_o[:, nb * 512 : (nb + 1) * 512],
                            lhsT,
                            wo_sb[:, nb, 2 * kcp : 2 * kcp + 2, :],
                            start=(kcp == 0),
                            stop=False,
                            perf_mode=DR,
                        )
                prefill[jch] = ps_o

            for jch in range(4):
                emit_prefill(jch)
            for jch in range(NJCH):
                ps_o = prefill.pop(jch)
                lhsT = multiT[NKC // 2 - 1][:, :, jch * 128 : (jch + 1) * 128]
                for nb in range(2):
                    mm(
                        ps_o[:, nb * 512 : (nb + 1) * 512],
                        lhsT,
                        wo_sb[:, nb, NKC - 2 : NKC, :],
                        start=False,
                        stop=True,
                        perf_mode=DR,
                    )
                # x = psum + residual, sum_t = rowsum(x), in one DVE pass
                x_sb = pC.tile([128, D], F32, tag="x", name=f"x{jch}")
                sum_t = pStats.tile([128, 1], F32, tag="sum", name=f"sum{jch}")
                # x = psum/WSCALE + residual (undo the Wo fp8 pre-scale),
                # sum_t = rowsum(x), in one DVE pass
                nc.vector.scalar_tensor_tensor(
                    out=x_sb[:],
                    in0=ps_o[:],
                    scalar=1.0 / WSCALE,
                    in1=tokres_sb[:, jch, :],
                    op0=mult,
                    op1=add,
                    accum_out=sum_t[:],
                )
                negmean = pStats.tile([128, 1], F32, tag="nm", name=f"nm{jch}")
                nc.vector.tensor_scalar_mul(negmean[:], sum_t[:], -1.0 / D)
                # Variance always on Scalar (Square w/ accum); the final scale
                # alternates ACT/DVE by jch parity -- balances the two queues
                # at the drain with fewer cross-engine hops per chain.
                scrap = pC.tile([128, D], BF, tag="scrap", name=f"scrap{jch}")
                ssq = pStats.tile([128, 1], F32, tag="ssq", name=f"ssq{jch}")
                nc.scalar.activation(
                    scrap[:], x_sb[:], Square, bias=negmean[:], accum_out=ssq[:]
                )
                std_t = pStats.tile([128, 1], F32, tag="std", name=f"std{jch}")
                nc.scalar.activation(std_t[:], ssq[:], Sqrt, bias=eps_sb[:], scale=1.0 / D)
                rstd = pStats.tile([128, 1], F32, tag="rstd", name=f"rstd{jch}")
                nc.vector.reciprocal(rstd[:], std_t[:])
                # (x - m) * rstd == x*rstd + (negmean*rstd), one fused op
                rstd_nm = pStats.tile([128, 1], F32, tag="rnm", name=f"rnm{jch}")
                nc.vector.tensor_tensor(rstd_nm[:], negmean[:], rstd[:], mult)
                out_sb = pC.tile([128, D], F32, tag="out", name=f"out{jch}")
                if jch % 3 == 0:
                    nc.scalar.activation(
                        out_sb[:],
                        x_sb[:],
                        mybir.ActivationFunctionType.Identity,
                        bias=rstd_nm[:],
                        scale=rstd[:],
                    )
                elif jch % 3 == 1:
                    nc.vector.tensor_scalar(
                        out=out_sb[:], in0=x_sb[:], scalar1=rstd[:],
                        scalar2=rstd_nm[:], op0=mult, op1=add,
                    )
                else:
                    # third lane: GpSimd is idle at the drain (SBUF-only op)
                    nc.gpsimd.tensor_scalar(
                        out=out_sb[:], in0=x_sb[:], scalar1=rstd[:],
                        scalar2=rstd_nm[:], op0=mult, op1=add,
                    )
                if apply_affine:
                    nc.gpsimd.tensor_tensor(out_sb[:], out_sb[:], gamma_sb[:], mult)
                    nc.gpsimd.tensor_tensor(out_sb[:], out_sb[:], beta_sb[:], add)
                nc.sync.dma_start(out_d[:, jch], out_sb[:])
                # second prefill wave once the first four STTs are emitted, so
                # the PE chain never parks on a not-yet-freed PSUM group
                if jch == 3:
                    for j2 in range(4, NJCH):
                        emit_prefill(j2)

    nc.compile()
    return nc


def _prep_inputs(tokens, Wq, Wk, Wv, Wo, gamma, beta):
    """Host-side layout prep. Returns per-core input maps. All tensors use
    block-major layouts so every kernel DMA is contiguous per partition:
    [p, blk, kc, cols] with contraction row index kc*128+p."""
    tokens = np.ascontiguousarray(np.asarray(tokens, dtype=np.float32))

    def blocks(a, ncols):  # [1024, N] -> [128, N//ncols, NKC, ncols]
        return np.ascontiguousarray(
            a.reshape(NKC, 128, a.shape[-1] // ncols, ncols).transpose(1, 2, 0, 3)
        )

    wq_all = blocks(
        (np.asarray(Wq).transpose(1, 0, 2).reshape(D, H * DK) * WSCALE).astype(FP8),
        128,
    )
    wk_all = blocks(
        (np.asarray(Wk).transpose(1, 0, 2).reshape(D, H * DK) * WSCALE).astype(FP8),
        128,
    )
    wv_all = blocks(
        (np.asarray(Wv).transpose(1, 0, 2).reshape(D, H * DV) * WSCALE).astype(FP8),
        512,
    )
    wo_all = blocks((np.asarray(Wo) * WSCALE).astype(FP8), 512)
    gamma_bc = np.ascontiguousarray(
        np.broadcast_to(np.asarray(gamma, np.float32), (128, D))
    )
    beta_bc = np.ascontiguousarray(
        np.broadcast_to(np.asarray(beta, np.float32), (128, D))
    )

    tokT_by_b = []
    for b in range(B):
        tokT_by_b.append(blocks(tokens[b].T.astype(FP8), 512))  # [128,4,8,512]

    in_maps = []
    for c in range(NCORES):
        b, jc = c // 2, c % 2
        tokT = tokT_by_b[b]
        tokTj = np.ascontiguousarray(tokT[:, 2 * jc : 2 * jc + 2])
        tokres = np.ascontiguousarray(
            tokens[b, jc * JW : (jc + 1) * JW]
            .reshape(NJCH, 128, D)
            .transpose(1, 0, 2)
        )
        in_maps.append(
            {
                "tokT": tokT,
                "tokTj": tokTj,
                "wq": wq_all,
                "wk": wk_all,
                "wv": wv_all,
                "wo": wo_all,
                "tokres": tokres,
                "gamma_bc": gamma_bc,
                "beta_bc": beta_bc,
            }
        )
    return in_maps


def run(inputs, trace=False, tmpdir=None):
    """Run on hardware; returns (output, BassKernelResults)."""
    from concourse.bass_utils import run_bass_kernel_spmd

    apply_affine = not (
        np.all(np.asarray(inputs["gamma"]) == 1.0)
        and np.all(np.asarray(inputs["beta"]) == 0.0)
    )
    key = ("nc", apply_affine)
    if key not in _CACHE:
        _CACHE[key] = _build_nc(apply_affine)
    nc = _CACHE[key]
    in_maps = _prep_inputs(**inputs)
    res = run_bass_kernel_spmd(
        nc, in_maps, core_ids=list(range(NCORES)), trace=trace, tmpdir=tmpdir
    )
    out = np.empty((B, S, D), np.float32)
    for c in range(NCORES):
        b, jc = c // 2, c % 2
        o = res.results[c]["out"]  # [128, 8, 1024]
        out[b, jc * JW : (jc + 1) * JW] = (
            o.transpose(1, 0, 2).reshape(JW, D)
        )
    return out, res


def kernel(tokens, Wq, Wk, Wv, Wo, gamma, beta):
    out, _ = run(
        dict(tokens=tokens, Wq=Wq, Wk=Wk, Wv=Wv, Wo=Wo, gamma=gamma, beta=beta)
    )
    return out

